# revision 1
# baseline (speedup 1.0000x reference)
"""Trainium2 Bass kernel for nn_ModelConTT_46016279609475 (TT interpolation).

y[b] = v0[b]^T V1[b] V2[b] v3[b], where v_i are linearly-interpolated slices
of tiny TT cores at per-point grid coordinates derived from x[b, :].

Strategy (per NeuronCore, data-parallel over B):
  * Precompute joint tables on device with PE matmuls:
      G[n0, n1, k] = sum_c core0[n0, c] * core1[c, n1, k]      (u-side)
      H[n3, n2, k] = sum_c core3[c, n3] * core2[k, n2, c]      (v-side)
    stored f32 in DRAM as 4-corner-packed 256B entries [dn0, dn1, k]:
      G4[(n0*128+n1), (dn0, dn1, k)] = G[n0+dn0, n1+dn1, k]
    so one dma_gather element fetches everything needed for the bilinear
    interpolation of u[b] (and same for v[b]).
  * Per point: idx = lo0*128 + lo1 (int16), one 256B dma_gather per table,
    DVE does the 4-corner weighted sum and the final k-dot:
      y[b] = sum_k (sum_c wG_c gG[c,k]) * (sum_c wH_c gH[c,k])

Batch mapping per core: shard b of size 32768; on-chip layout is
"p-minor": element i lives at partition i%128, free col i//128, matching
dma_gather's output layout dst[i%128, i//128]. Index lists are mod-16
wrapped as dma_gather requires (idx for i at [i%16, i//16]) and replicated
across all 8 Q7 core groups (each SWDGE core pair reads its own 16 rows).

Exact-floor trick (f32-safe): t = (xc + 2^23) - 2^23 rounds to nearest;
g = (t > xc); floor = t - g; frac = xc - floor computed via the exact
(t1 - 2^23) path to avoid re-rounding.
"""

import numpy as np
import ml_dtypes

import concourse.bass as bass
import concourse.bacc as bacc
import concourse.mybir as mybir
import concourse.tile as tile
from concourse import library_config
from concourse.bass_utils import run_bass_kernel_spmd

F32 = mybir.dt.float32
BF16 = mybir.dt.bfloat16
I16 = mybir.dt.int16
OP = mybir.AluOpType
AF = mybir.ActivationFunctionType

NCORES = 8
B = 262144
BS = B // NCORES          # 32768 points per core
P = 128                   # partitions
J = BS // P               # 256 free cols per partition
NCH = 8                   # pipeline chunks
JC = J // NCH             # 32 cols per chunk
NIDX = P * JC             # 4096 idxs per gather
LC = NIDX // 16           # 256 idx-list cols per chunk
N = 128                   # mode size
R = 16                    # TT rank
TE = N * N                # table entries
ES = 64                   # f32 elems per entry: 4 corners x 16 k = 256B
MAGIC = float(2 ** 23)
SCALE = (N - 1) / 2.0     # 63.5
M16 = BS // 16            # 2048 idx-list cols total

_CACHED = None
DEBUG_TILES = {}


def _build_nc(stage="full"):
    nc = bacc.Bacc("TRN2")

    x_pm = nc.dram_tensor("x_pm", [P, J, 4], F32, kind="ExternalInput")
    xq = nc.dram_tensor("xq", [64, M16 // 2, 2], F32, kind="ExternalInput")
    c0t = nc.dram_tensor("c0t", [16, 129], F32, kind="ExternalInput")
    c1f = nc.dram_tensor("c1f", [16, 2096], F32, kind="ExternalInput")
    c3f = nc.dram_tensor("c3f", [16, 129], F32, kind="ExternalInput")
    c2t = nc.dram_tensor("c2t", [16, 2096], F32, kind="ExternalInput")
    y_pm = nc.dram_tensor("y_pm", [P, J], F32, kind="ExternalOutput")

    with tile.TileContext(nc) as tc:
        with (
            tc.tile_pool(name="per", bufs=1) as pe,
            tc.tile_pool(name="ps", bufs=4, space="PSUM") as pp,
            tc.tile_pool(name="dr", bufs=1, space="DRAM") as dp,
        ):
            nc.gpsimd.load_library(library_config.mlp)

            # persistent tiles (lists fully memset once: the gather idx AP
            # spans all 128 partitions but HW only reads rows 0-31, its
            # queue's core pair; the sim reads rows 0-15)
            LG = pe.tile([P, M16], I16)
            LH = pe.tile([P, M16], I16)
            nc.vector.memset(LG[:], 0)
            nc.scalar.memzero(LH[:])
            WG = pe.tile([P, 4, J], F32)
            WH = pe.tile([P, 4, J], F32)
            G4 = pe.tile([P, N, ES], F32)
            H4 = pe.tile([P, N, ES], F32)
            ysb = pe.tile([P, J], F32)
            g4d = dp.tile([TE, ES], F32)
            h4d = dp.tile([TE, ES], F32)

            with tc.tile_pool(name="pre", bufs=1) as wp:
                # ------------- constant loads -------------
                c0t_s = wp.tile([16, 129], F32)
                nc.sync.dma_start(c0t_s[:], c0t[:])
                c1f_s = wp.tile([16, 2096], F32)
                nc.sync.dma_start(c1f_s[:], c1f[:])
                c3f_s = wp.tile([16, 129], F32)
                nc.sync.dma_start(c3f_s[:], c3f[:])
                c2t_s = wp.tile([16, 2096], F32)
                nc.sync.dma_start(c2t_s[:], c2t[:])
                x_s = wp.tile([P, J * 4], F32)
                nc.sync.dma_start(x_s[:], x_pm[:].rearrange("p a b -> p (a b)"))
                xq_s = wp.tile([112, M16], F32)
                nc.vector.memset(xq_s[:], 0.0)
                xqv = xq[:].rearrange("p a b -> p (a b)")
                nc.sync.dma_start(xq_s[0:16, :], xqv[0:16, :])
                nc.sync.dma_start(xq_s[32:48, :], xqv[16:32, :])
                nc.sync.dma_start(xq_s[64:80, :], xqv[32:48, :])
                nc.sync.dma_start(xq_s[96:112, :], xqv[48:64, :])

                # ------------- table build ----------------
                # chunk-outer so each n1-quarter's DRAM write starts as soon
                # as its four corner copies land (overlaps write with build).
                for tbl, tdr, lhs, rhs in (
                    (G4, g4d, c0t_s, c1f_s),
                    (H4, h4d, c3f_s, c2t_s),
                ):
                    tblv = tbl[:].rearrange("p n (c k) -> p n c k", k=R)
                    tdrv = tdr[:].rearrange("(p a) b -> p (a b)", p=P)
                    tsbv = tbl[:].rearrange("p a b -> p (a b)")
                    for ch in range(4):
                        for ci, (dhi, dlo) in enumerate(
                            ((0, 0), (0, 1), (1, 0), (1, 1))
                        ):
                            ps = pp.tile([P, 512], F32, tag="mmps")
                            nc.tensor.matmul(
                                ps[:],
                                lhs[:, dhi : dhi + 128],
                                rhs[
                                    :,
                                    16 * dlo + 512 * ch : 16 * dlo + 512 * ch + 512,
                                ],
                                start=True,
                                stop=True,
                            )
                            dst = tblv[:, 32 * ch : 32 * ch + 32, ci, :]
                            src = ps[:].rearrange("p (a b) -> p a b", b=R)
                            if ci % 2 == 0:
                                nc.vector.tensor_copy(dst, src)
                            else:
                                nc.scalar.copy(dst, src)
                        nc.sync.dma_start(
                            tdrv[:, 2048 * ch : 2048 * ch + 2048],
                            tsbv[:, 2048 * ch : 2048 * ch + 2048],
                        )

                # ------------- index lists ----------------
                # four 16-row bands (at partition bases 0/32/64/96 -- the
                # only legal compute starts): G cols 0-1023 / G cols
                # 1024-2047 / H cols 0-1023 / H cols 1024-2047. Halves the
                # per-op free size vs a single band.
                nc.vector.tensor_scalar(
                    xq_s[:], xq_s[:], SCALE, SCALE, OP.mult, OP.add
                )
                t1q = wp.tile([112, M16], F32)
                nc.scalar.activation(t1q[:], xq_s[:], AF.Copy, bias=MAGIC, scale=1.0)
                gq = wp.tile([112, M16], F32)
                nc.vector.scalar_tensor_tensor(
                    gq[:], t1q[:], -MAGIC, xq_s[:], OP.add, OP.is_gt
                )
                # lo = (t1 - MAGIC) - g  (exact floor), in place over t1q
                nc.vector.scalar_tensor_tensor(
                    t1q[:], t1q[:], -MAGIC, gq[:], OP.add, OP.subtract
                )
                # idx = lo_hi*128 + lo_lo, int16 cast fused into the op's
                # output dtype; written straight into the list tiles.
                lo_hi = t1q[:].rearrange("p (m two) -> p m two", two=2)
                H16 = M16 // 2
                for band, dst in (
                    (0, LG[0:16, 0:H16]),
                    (32, LG[32:48, H16:M16]),
                    (64, LH[64:80, 0:H16]),
                    (96, LH[96:112, H16:M16]),
                ):
                    nc.vector.scalar_tensor_tensor(
                        dst,
                        lo_hi[band : band + 16, :, 0],
                        128.0,
                        lo_hi[band : band + 16, :, 1],
                        OP.mult,
                        OP.add,
                    )
                nc.sync.dma_start(LG[0:16, H16:M16], LG[32:48, H16:M16])
                nc.sync.dma_start(LG[16:32, :], LG[0:16, :])
                nc.sync.dma_start(LH[0:16, 0:H16], LH[64:80, 0:H16])
                nc.sync.dma_start(LH[0:16, H16:M16], LH[96:112, H16:M16])
                nc.sync.dma_start(LH[16:32, :], LH[0:16, :])

                # ------------- interp weights -------------
                # x_s is [128, (256 j, 4 d)]; w = frac(xc), a = 1 - w.
                nc.vector.tensor_scalar(
                    x_s[:], x_s[:], SCALE, SCALE, OP.mult, OP.add
                )
                t1 = wp.tile([P, J * 4], F32)
                nc.scalar.activation(t1[:], x_s[:], AF.Copy, bias=MAGIC, scale=1.0)
                gw = wp.tile([P, J * 4], F32)
                nc.vector.scalar_tensor_tensor(
                    gw[:], t1[:], -MAGIC, x_s[:], OP.add, OP.is_gt
                )
                # s1 = (t1 - MAGIC) - xc = t - xc   (exact: t1 - MAGIC is exact)
                s1 = wp.tile([P, J * 4], F32)
                nc.vector.scalar_tensor_tensor(
                    s1[:], t1[:], -MAGIC, x_s[:], OP.add, OP.subtract
                )
                # w = g - (t - xc) = xc - floor(xc), in place over s1
                nc.vector.tensor_tensor(s1[:], gw[:], s1[:], OP.subtract)
                aw = wp.tile([P, J * 4], F32, tag="t1")
                nc.vector.tensor_scalar(aw[:], s1[:], -1.0, 1.0, OP.mult, OP.add)

                wv = s1[:].rearrange("p (j d) -> p j d", d=4)
                av = aw[:].rearrange("p (j d) -> p j d", d=4)
                # G corners (dn0, dn1): (a0,a1),(a0,w1),(w0,a1),(w0,w1)
                nc.vector.tensor_tensor(WG[:, 0, :], av[:, :, 0], av[:, :, 1], OP.mult)
                nc.vector.tensor_tensor(WG[:, 1, :], av[:, :, 0], wv[:, :, 1], OP.mult)
                nc.vector.tensor_tensor(WG[:, 2, :], wv[:, :, 0], av[:, :, 1], OP.mult)
                nc.vector.tensor_tensor(WG[:, 3, :], wv[:, :, 0], wv[:, :, 1], OP.mult)
                # H corners (dn3, dn2): (a3,a2),(a3,w2),(w3,a2),(w3,w2)
                nc.vector.tensor_tensor(WH[:, 0, :], av[:, :, 3], av[:, :, 2], OP.mult)
                nc.vector.tensor_tensor(WH[:, 1, :], av[:, :, 3], wv[:, :, 2], OP.mult)
                nc.vector.tensor_tensor(WH[:, 2, :], wv[:, :, 3], av[:, :, 2], OP.mult)
                nc.vector.tensor_tensor(WH[:, 3, :], wv[:, :, 3], wv[:, :, 2], OP.mult)

            # ------------- gather + combine ---------------
            if stage != "full":
                nc.vector.memset(ysb[:], 0.0)
            nch = {"tables": 0, "gather1": 1}.get(stage, NCH)
            with (
                tc.tile_pool(name="gbuf", bufs=3) as gb,
                tc.tile_pool(name="cbuf", bufs=2) as cb,
            ):
                for ch in range(nch):
                    gGt = gb.tile([P, JC, ES], F32, tag="gG")
                    nc.gpsimd.dma_gather(
                        gGt[:],
                        g4d[:],
                        LG[:, LC * ch : LC * ch + LC],
                        NIDX,
                        NIDX,
                        ES,
                        queue_num=0,
                        single_packet=False,
                    )
                    gHt = gb.tile([P, JC, ES], F32, tag="gH")
                    nc.gpsimd.dma_gather(
                        gHt[:],
                        h4d[:],
                        LH[:, LC * ch : LC * ch + LC],
                        NIDX,
                        NIDX,
                        ES,
                        queue_num=0,
                        single_packet=False,
                    )
                    gG = gGt[:]
                    gH = gHt[:]

                    uv = []
                    for ti, (g, W) in enumerate(((gG, WG), (gH, WH))):
                        # m[c, j, k] = gathered corner value * corner weight
                        # (weight broadcast over k via stride-0 AP; no
                        # materialization)
                        m = cb.tile([P, 4, JC, R], F32, tag=f"m{ti}")
                        gv = g.rearrange("p j (c k) -> p c j k", c=4)
                        wbc = (
                            W[:, :, JC * ch : JC * ch + JC]
                            .unsqueeze(3)
                            .broadcast_to([P, 4, JC, R])
                        )
                        nc.vector.tensor_tensor(m[:], gv, wbc, OP.mult)
                        t2 = cb.tile([P, 2, JC, R], F32, tag=f"t{ti}")
                        nc.vector.tensor_tensor(
                            t2[:], m[:, 0:2], m[:, 2:4], OP.add
                        )
                        u = cb.tile([P, JC, R], F32, tag=f"u{ti}")
                        nc.vector.tensor_tensor(u[:], t2[:, 0], t2[:, 1], OP.add)
                        uv.append(u)

                    pr = cb.tile([P, JC, R], F32, tag="pr")
                    nc.vector.tensor_tensor(pr[:], uv[0][:], uv[1][:], OP.mult)
                    nc.vector.tensor_reduce(
                        ysb[:, JC * ch : JC * ch + JC],
                        pr[:],
                        mybir.AxisListType.X,
                        OP.add,
                    )

            nc.sync.dma_start(y_pm[:], ysb[:])
            DEBUG_TILES.update(LG=LG, LH=LH, WG=WG, WH=WH, G4=G4, H4=H4,
                               ysb=ysb, g4d=g4d, h4d=h4d)

    nc.finalize()
    return nc


def _prep_inputs(x, core0, core1, core2, core3):
    """Host-side input marshalling: shard x over cores, lay out tensors in
    the on-chip layouts the kernel expects, pad core matrices for the
    shifted-corner matmuls (cast to bf16 on host)."""
    xs = np.ascontiguousarray(np.asarray(x, dtype=np.float32).reshape(NCORES, BS, 4))

    core0 = np.asarray(core0, dtype=np.float32)
    core1 = np.asarray(core1, dtype=np.float32)
    core2 = np.asarray(core2, dtype=np.float32)
    core3 = np.asarray(core3, dtype=np.float32)

    c0 = core0[0]                        # [128, 16]
    c0t = np.ascontiguousarray(np.concatenate([c0.T, c0.T[:, -1:]], axis=1))
    c1 = core1.reshape(16, 2048)
    c1f = np.ascontiguousarray(
        np.concatenate([c1, np.tile(c1[:, -16:], (1, 3))], axis=1)
    )
    c2 = np.ascontiguousarray(core2.transpose(2, 1, 0)).reshape(16, 2048)
    c2t = np.ascontiguousarray(
        np.concatenate([c2, np.tile(c2[:, -16:], (1, 3))], axis=1)
    )
    c3 = core3[:, :, 0]                  # [16, 128]
    c3f = np.ascontiguousarray(np.concatenate([c3, c3[:, -1:]], axis=1))

    in_maps = []
    for c in range(NCORES):
        xc_ = xs[c]
        x_pm = np.ascontiguousarray(
            xc_.reshape(J, P, 4).transpose(1, 0, 2)
        )  # [128, 256, 4]
        xg = np.ascontiguousarray(
            xc_[:, [0, 1]].reshape(M16, 16, 2).transpose(1, 0, 2)
        )  # [16, 2048, 2]
        xh = np.ascontiguousarray(
            xc_[:, [3, 2]].reshape(M16, 16, 2).transpose(1, 0, 2)
        )
        H16 = M16 // 2
        xq = np.concatenate(
            [xg[:, :H16], xg[:, H16:], xh[:, :H16], xh[:, H16:]], axis=0
        )  # [64, 1024, 2]
        in_maps.append(
            {
                "x_pm": x_pm,
                "xq": xq,
                "c0t": c0t,
                "c1f": c1f,
                "c3f": c3f,
                "c2t": c2t,
            }
        )
    return in_maps


def kernel(x, core0, core1, core2, core3):
    global _CACHED
    if _CACHED is None:
        _CACHED = _build_nc()
    nc = _CACHED
    in_maps = _prep_inputs(x, core0, core1, core2, core3)
    res = run_bass_kernel_spmd(nc, in_maps, core_ids=list(range(NCORES)))
    outs = []
    for c in range(NCORES):
        y_pm = res.results[c]["y_pm"]          # [128, 256]
        outs.append(np.ascontiguousarray(np.asarray(y_pm).T).reshape(-1))
    return np.concatenate(outs).astype(np.float32)



# revision 2
# speedup vs baseline: 1.1495x; 1.1495x over previous
"""Trainium2 Bass kernel for nn_ModelConTT_46016279609475 (TT interpolation).

y[b] = v0[b]^T V1[b] V2[b] v3[b], where v_i are linearly-interpolated slices
of tiny TT cores at per-point grid coordinates derived from x[b, :].

Strategy (per NeuronCore, data-parallel over B):
  * The joint pair tables are built ON HOST (weight preprocessing, O(N^2 R^2)
    one-time work independent of B):
      G[n0, n1, k] = sum_c core0[n0, c] * core1[c, n1, k]      (u-side)
      H[n3, n2, k] = sum_c core3[c, n3] * core2[k, n2, c]      (v-side)
    packed 4-corner per cell in f16 (64 useful + 64 pad values = 256 B, the
    minimum legal dma_gather element):
      TAB[(n*128+m), (dn*2+dm)*16 + k] = T[n+dn, m+dm, k]
    so one dma_gather element fetches everything needed for the bilinear
    interpolation of u[b] (and same for v[b]).
  * Device: compute per-point cell ids (int16) + corner weights (f16) from x,
    then 2 dma_gathers per chunk (one per table) and a DVE combine:
      y[b] = sum_k (sum_c wG_c gG[c,k]) * (sum_c wH_c gH[c,k])

Batch mapping per core: shard b of size 32768; on-chip layout is "p-minor":
element i lives at partition i%128, free col i//128, matching dma_gather's
output layout dst[i%128, i//128]. Index lists are mod-16 wrapped as
dma_gather requires (idx for i at [i%16, i//16]) and replicated to rows
16-31 (the queue's core pair); rows 32-127 are zeroed on the Pool engine so
the full-height index AP passes validation.

Exact-floor trick (f32-safe): t = (xc + 2^23) - 2^23 rounds to nearest;
g = (t > xc); floor = t - g; frac = xc - floor computed via the exact
(t1 - 2^23) path to avoid re-rounding.
"""

import numpy as np

import concourse.bass as bass
import concourse.bacc as bacc
import concourse.mybir as mybir
import concourse.tile as tile
from concourse import library_config
from concourse.bass_utils import run_bass_kernel_spmd

F32 = mybir.dt.float32
F16 = mybir.dt.float16
I16 = mybir.dt.int16
OP = mybir.AluOpType
AF = mybir.ActivationFunctionType

NCORES = 8
B = 262144
BS = B // NCORES          # 32768 points per core
P = 128                   # partitions
J = BS // P               # 256 free cols per partition
NCH = 8                   # pipeline chunks
JC = J // NCH             # 32 cols per chunk
NIDX = P * JC             # 4096 idxs per gather
LC = NIDX // 16           # 256 idx-list cols per chunk per table
N = 128                   # mode size
R = 16                    # TT rank
TE = N * N                # table entries per side
ES = 128                  # f16 elems per gather element (64 useful + 64 pad)
MAGIC = float(2 ** 23)
SCALE = (N - 1) / 2.0     # 63.5
M16 = BS // 16            # 2048 idx-list cols total (per side: 1024)

_CACHED = None


def _build_nc():
    nc = bacc.Bacc("TRN2")

    x_pm = nc.dram_tensor("x_pm", [P, J, 4], F32, kind="ExternalInput")
    xq = nc.dram_tensor("xq", [64, M16 // 2, 2], F32, kind="ExternalInput")
    tab = nc.dram_tensor("tab", [2 * TE, ES], F16, kind="ExternalInput")
    y_pm = nc.dram_tensor("y_pm", [P, J], F32, kind="ExternalOutput")

    tabG = tab[0:TE, :]
    tabH = tab[TE : 2 * TE, :]

    with tile.TileContext(nc) as tc:
        with tc.tile_pool(name="per", bufs=1) as pe:
            nc.gpsimd.load_library(library_config.mlp)

            # persistent tiles. LL holds both tables' index lists, laid out
            # [128, ch, side, LC]: per chunk a G block then an H block, each
            # a contiguous LC-col run (so each gather's idx AP is contiguous).
            LL = pe.tile([P, NCH, 2, LC], I16)
            nc.gpsimd.memset(LL[:], 0)
            Wg = pe.tile([P, 4, J], F16)
            Wh = pe.tile([P, 4, J], F16)
            ysb = pe.tile([P, J], F32)

            with tc.tile_pool(name="pre", bufs=1) as wp:
                # ---------------- index lists ----------------
                # xq holds (hi, lo) coordinate pairs in four 16-row bands at
                # partition bases 0/32/64/96 (legal compute starts): G chunks
                # 0-3 / G chunks 4-7 / H chunks 0-3 / H chunks 4-7.
                xq_s = wp.tile([112, M16], F32)
                nc.gpsimd.memset(xq_s[:], 0.0)
                xqv = xq[:].rearrange("p a b -> p (a b)")
                nc.sync.dma_start(xq_s[0:16, :], xqv[0:16, :])
                nc.sync.dma_start(xq_s[32:48, :], xqv[16:32, :])
                nc.sync.dma_start(xq_s[64:80, :], xqv[32:48, :])
                nc.sync.dma_start(xq_s[96:112, :], xqv[48:64, :])

                # xc = 63.5*x + 63.5 in place; t1 = xc + MAGIC (both on Act)
                nc.scalar.activation(
                    xq_s[:], xq_s[:], AF.Copy, bias=SCALE, scale=SCALE
                )
                t1q = wp.tile([112, M16], F32)
                nc.scalar.activation(t1q[:], xq_s[:], AF.Copy, bias=MAGIC, scale=1.0)
                gq = wp.tile([112, M16], F32)
                nc.vector.scalar_tensor_tensor(
                    gq[:], t1q[:], -MAGIC, xq_s[:], OP.add, OP.is_gt
                )
                # lo = (t1 - MAGIC) - g  (exact floor), in place over t1q
                nc.vector.scalar_tensor_tensor(
                    t1q[:], t1q[:], -MAGIC, gq[:], OP.add, OP.subtract
                )
                # idx = lo_hi*128 + lo_lo for all four bands in one op,
                # int16 cast fused into the output dtype.
                lo_pair = t1q[:].rearrange("p (m two) -> p m two", two=2)
                stage = wp.tile([112, M16 // 2], I16)
                nc.vector.scalar_tensor_tensor(
                    stage[:],
                    lo_pair[:, :, 0],
                    128.0,
                    lo_pair[:, :, 1],
                    OP.mult,
                    OP.add,
                )
                # marshal bands into LL rows 0-15 and replicate to 16-31
                H16 = M16 // 2
                Q16 = M16 // 4
                for band, ch0, side in (
                    (0, 0, 0),
                    (32, 4, 0),
                    (64, 0, 1),
                    (96, 4, 1),
                ):
                    src = stage[band : band + 16, :].rearrange(
                        "p (c l) -> p c l", l=LC
                    )
                    nc.sync.dma_start(LL[0:16, ch0 : ch0 + 4, side, :], src)
                nc.sync.dma_start(
                    LL[16:32].rearrange("p a b c -> p (a b c)"),
                    LL[0:16].rearrange("p a b c -> p (a b c)"),
                )

                # ---------------- interp weights ----------------
                # x_s is [128, (256 j, 4 d)]; w = frac(xc), a = 1 - w, stored
                # planar f16 in aw = [128, (a|w), 4 d, 256 j] so downstream
                # ops keep stride-1 innermost dims (DVE 2x/4x perf modes).
                x_s = wp.tile([P, J * 4], F32)
                nc.sync.dma_start(x_s[:], x_pm[:].rearrange("p a b -> p (a b)"))
                nc.scalar.activation(x_s[:], x_s[:], AF.Copy, bias=SCALE, scale=SCALE)
                t1 = wp.tile([P, J * 4], F32)
                nc.scalar.activation(t1[:], x_s[:], AF.Copy, bias=MAGIC, scale=1.0)
                gw = wp.tile([P, J * 4], F32)
                nc.vector.scalar_tensor_tensor(
                    gw[:], t1[:], -MAGIC, x_s[:], OP.add, OP.is_gt
                )
                # s1 = (t1 - MAGIC) - xc = t - xc   (exact: t1 - MAGIC is exact)
                s1 = wp.tile([P, J * 4], F32)
                nc.vector.scalar_tensor_tensor(
                    s1[:], t1[:], -MAGIC, x_s[:], OP.add, OP.subtract
                )
                aw = wp.tile([P, 2, 4, J], F16)
                # w = g - (t - xc) = xc - floor(xc); write planar-transposed
                gv = gw[:].rearrange("p (j d) -> p d j", d=4)
                sv = s1[:].rearrange("p (j d) -> p d j", d=4)
                nc.vector.tensor_tensor(aw[:, 1], gv, sv, OP.subtract)
                # a = 1 - w (f16 all through, 4x-eligible)
                nc.vector.tensor_scalar(
                    aw[:, 0], aw[:, 1], -1.0, 1.0, OP.mult, OP.add
                )
                # corner products: c order = (hi,lo) in {(a,a),(a,w),(w,a),(w,w)}
                # G uses dims (0 hi, 1 lo); H uses dims (3 hi, 2 lo).
                for W, dhi, dlo in ((Wg, 0, 1), (Wh, 3, 2)):
                    in1 = aw[:, :, dlo, :]  # [P, (a|w), J]
                    for ci, sel in ((0, 0), (2, 1)):
                        in0 = aw[:, sel, dhi, :].unsqueeze(1).broadcast_to(
                            [P, 2, J]
                        )
                        nc.vector.tensor_tensor(
                            W[:, ci : ci + 2, :], in0, in1, OP.mult
                        )

            # ---------------- gather + combine ----------------
            with (
                tc.tile_pool(name="gbuf", bufs=3) as gb,
                tc.tile_pool(name="cbuf", bufs=2) as cb,
            ):
                for ch in range(NCH):
                    gG = gb.tile([P, JC, ES], F16, tag="gG")
                    nc.gpsimd.dma_gather(
                        gG[:],
                        tabG,
                        LL[:, ch, 0, :],
                        NIDX,
                        NIDX,
                        ES,
                        queue_num=0,
                        single_packet=False,
                    )
                    gH = gb.tile([P, JC, ES], F16, tag="gH")
                    nc.gpsimd.dma_gather(
                        gH[:],
                        tabH,
                        LL[:, ch, 1, :],
                        NIDX,
                        NIDX,
                        ES,
                        queue_num=0,
                        single_packet=False,
                    )

                    uv = []
                    for ti, (g, W) in enumerate(((gG, Wg), (gH, Wh))):
                        # m[c, j, k] = gathered corner value * corner weight
                        # (weight broadcast over k via stride-0 AP)
                        m = cb.tile([P, 4, JC, R], F16, tag=f"m{ti}")
                        gv2 = g[:, :, 0 : 4 * R].rearrange(
                            "p j (c k) -> p c j k", k=R
                        )
                        wbc = (
                            W[:, :, JC * ch : JC * ch + JC]
                            .unsqueeze(3)
                            .broadcast_to([P, 4, JC, R])
                        )
                        nc.vector.tensor_tensor(m[:], gv2, wbc, OP.mult)
                        t2 = cb.tile([P, 2, JC, R], F16, tag=f"t{ti}")
                        nc.vector.tensor_tensor(
                            t2[:], m[:, 0:2], m[:, 2:4], OP.add
                        )
                        u = cb.tile([P, JC, R], F16, tag=f"u{ti}")
                        nc.vector.tensor_tensor(u[:], t2[:, 0], t2[:, 1], OP.add)
                        uv.append(u)

                    pr = cb.tile([P, JC, R], F16, tag="pr")
                    nc.vector.tensor_tensor(pr[:], uv[0][:], uv[1][:], OP.mult)
                    nc.vector.tensor_reduce(
                        ysb[:, JC * ch : JC * ch + JC],
                        pr[:],
                        mybir.AxisListType.X,
                        OP.add,
                    )

            nc.sync.dma_start(y_pm[:], ysb[:])

    nc.finalize()
    return nc


def _pack_table(T):
    """T [128, 128, 16] f32 -> [16384, 128] f16, 4-corner packed + pad."""
    ar = np.arange(N)
    out = np.zeros((N, N, ES), dtype=np.float16)
    for ci, (dn, dm) in enumerate(((0, 0), (0, 1), (1, 0), (1, 1))):
        rn = np.minimum(ar + dn, N - 1)
        rm = np.minimum(ar + dm, N - 1)
        out[:, :, ci * R : ci * R + R] = T[np.ix_(rn, rm)].astype(np.float16)
    return np.ascontiguousarray(out.reshape(TE, ES))


def _prep_inputs(x, core0, core1, core2, core3):
    """Host-side input marshalling: shard x over cores, lay out tensors in
    the on-chip layouts the kernel expects, and precompute the 4-corner
    packed f16 pair tables (weight preprocessing, O(N^2 R^2))."""
    xs = np.ascontiguousarray(np.asarray(x, dtype=np.float32).reshape(NCORES, BS, 4))

    core0 = np.asarray(core0, dtype=np.float32)
    core1 = np.asarray(core1, dtype=np.float32)
    core2 = np.asarray(core2, dtype=np.float32)
    core3 = np.asarray(core3, dtype=np.float32)

    # G[n0, n1, k] = sum_c core0[0, n0, c] core1[c, n1, k]
    G = np.einsum("nc,cmk->nmk", core0[0], core1, optimize=True)
    # H[n3, n2, k] = sum_c core3[c, n3, 0] core2[k, n2, c]
    H = np.einsum("cn,kmc->nmk", core3[:, :, 0], core2, optimize=True)
    tab = np.concatenate([_pack_table(G), _pack_table(H)], axis=0)

    in_maps = []
    for c in range(NCORES):
        xc_ = xs[c]
        x_pm = np.ascontiguousarray(
            xc_.reshape(J, P, 4).transpose(1, 0, 2)
        )  # [128, 256, 4]
        # band source for index lists: point i of chunk ch sits at list
        # position [(i%16), ch*LC + i//16] of its side; bands cover 4 chunks
        # each: [16 rows, 4*LC cols] per band, bands stacked to [64, 1024, 2]
        xg = np.ascontiguousarray(
            xc_[:, [0, 1]].reshape(NCH, LC, 16, 2).transpose(0, 2, 1, 3)
        )  # [ch, 16, LC, 2]
        xh = np.ascontiguousarray(
            xc_[:, [3, 2]].reshape(NCH, LC, 16, 2).transpose(0, 2, 1, 3)
        )
        xq = np.concatenate(
            [
                xg[0:4].transpose(1, 0, 2, 3).reshape(16, 4 * LC, 2),
                xg[4:8].transpose(1, 0, 2, 3).reshape(16, 4 * LC, 2),
                xh[0:4].transpose(1, 0, 2, 3).reshape(16, 4 * LC, 2),
                xh[4:8].transpose(1, 0, 2, 3).reshape(16, 4 * LC, 2),
            ],
            axis=0,
        )  # [64, 1024, 2]
        in_maps.append(
            {
                "x_pm": x_pm,
                "xq": np.ascontiguousarray(xq),
                "tab": tab,
            }
        )
    return in_maps


def kernel(x, core0, core1, core2, core3):
    global _CACHED
    if _CACHED is None:
        _CACHED = _build_nc()
    nc = _CACHED
    in_maps = _prep_inputs(x, core0, core1, core2, core3)
    res = run_bass_kernel_spmd(nc, in_maps, core_ids=list(range(NCORES)))
    outs = []
    for c in range(NCORES):
        y_pm = res.results[c]["y_pm"]          # [128, 256]
        outs.append(np.ascontiguousarray(np.asarray(y_pm).T).reshape(-1))
    return np.concatenate(outs).astype(np.float32)


# revision 5
# speedup vs baseline: 1.2172x; 1.0589x over previous
"""Trainium2 Bass kernel for nn_ModelConTT_46016279609475 (TT interpolation).

y[b] = v0[b]^T V1[b] V2[b] v3[b], where v_i are linearly-interpolated slices
of tiny TT cores at per-point grid coordinates derived from x[b, :].

Strategy (per NeuronCore, data-parallel over B):
  * The joint pair tables are built ON HOST (weight preprocessing, O(N^2 R^2)
    one-time work independent of B):
      G[n0, n1, k] = sum_c core0[n0, c] * core1[c, n1, k]      (u-side)
      H[n3, n2, k] = sum_c core3[c, n3] * core2[k, n2, c]      (v-side)
    packed 4-corner per cell in f16 (64 useful + 64 pad values = 256 B, the
    minimum legal dma_gather element):
      TAB[(n*128+m), (dn*2+dm)*16 + k] = T[n+dn, m+dm, k]
    so one dma_gather element fetches everything needed for the bilinear
    interpolation of u[b] (and same for v[b]).
  * Device: compute per-point cell ids (int16) + corner weights (f16) from x,
    then 2 dma_gathers per chunk (one per table) and a DVE combine:
      y[b] = sum_k (sum_c wG_c gG[c,k]) * (sum_c wH_c gH[c,k])

Batch mapping per core: shard b of size 32768; on-chip layout is "p-minor":
element i lives at partition i%128, free col i//128, matching dma_gather's
output layout dst[i%128, i//128]. Index lists are mod-16 wrapped as
dma_gather requires (idx for i at [i%16, i//16]) and replicated to rows
16-31 (the queue's core pair); rows 32-127 are zeroed on the Pool engine so
the full-height index AP passes validation. The index math runs on a
[128, 512]-shaped coordinate tile (8 row-blocks of 16 rows, one per pair of
chunks) so every partition carries real data; small per-block DMAs then
marshal the int16 lists into the wrapped layout.

Exact-floor trick (f32-safe): t = (xc + 2^23) - 2^23 rounds to nearest;
g = (t > xc); floor = t - g; frac = xc - floor computed via the exact
(t1 - 2^23) path to avoid re-rounding.
"""

import numpy as np

import concourse.bass as bass
import concourse.bacc as bacc
import concourse.mybir as mybir
import concourse.tile as tile
from concourse import library_config
from concourse.bass_utils import run_bass_kernel_spmd

F32 = mybir.dt.float32
F16 = mybir.dt.float16
I16 = mybir.dt.int16
OP = mybir.AluOpType
AF = mybir.ActivationFunctionType

NCORES = 8
B = 262144
BS = B // NCORES          # 32768 points per core
P = 128                   # partitions
J = BS // P               # 256 free cols per partition
NCH = 16                  # pipeline chunks
JC = J // NCH             # 16 cols per chunk
NIDX = P * JC             # 2048 idxs per gather
LC = NIDX // 16           # 128 idx-list cols per chunk per table
N = 128                   # mode size
R = 16                    # TT rank
TE = N * N                # table entries per side
ES = 128                  # f16 elems per gather element (64 useful + 64 pad)
MAGIC = float(2 ** 23)
SCALE = (N - 1) / 2.0     # 63.5
NB = 8                    # row-blocks in the coordinate tile
CB = 2 * BS // 16 // NB   # 512 pair-cols per row (G+H, 2 chunks)

_CACHED = None


def _build_nc():
    nc = bacc.Bacc("TRN2")

    x_pm = nc.dram_tensor("x_pm", [P, J, 4], F32, kind="ExternalInput")
    xq = nc.dram_tensor("xq", [P, CB, 2], F32, kind="ExternalInput")
    tab = nc.dram_tensor("tab", [2 * TE, ES], F16, kind="ExternalInput")
    y_pm = nc.dram_tensor("y_pm", [P, J], F32, kind="ExternalOutput")

    tabG = tab[0:TE, :]
    tabH = tab[TE : 2 * TE, :]

    with tile.TileContext(nc) as tc:
        with tc.tile_pool(name="per", bufs=1) as pe:
            nc.gpsimd.load_library(library_config.mlp)

            # persistent tiles. LL holds both tables' index lists, laid out
            # [128, ch, side, LC]: per chunk a G block then an H block, each
            # a contiguous LC-col run (so each gather's idx AP is contiguous).
            LL = pe.tile([P, NCH, 2, LC], I16)
            nc.gpsimd.memset(LL[:], 0)
            Wg = pe.tile([P, 4, J], F16)
            Wh = pe.tile([P, 4, J], F16)
            ysb = pe.tile([P, J], F32)

            with tc.tile_pool(name="pre", bufs=1) as wp:
                # ---------------- index lists ----------------
                # xq holds (hi, lo) coordinate pairs; row-block rb (16 rows)
                # covers chunks 2rb..2rb+1 worth of list columns in wrapped
                # order: [G ch0 | H ch0 | G ch1 | H ch1] for that block.
                xq_s = wp.tile([P, 2 * CB], F32)
                nc.sync.dma_start(xq_s[:], xq[:].rearrange("p a b -> p (a b)"))
                # xc = 63.5*x + 63.5 in place; t1 = xc + MAGIC (both on Act)
                nc.scalar.activation(
                    xq_s[:], xq_s[:], AF.Copy, bias=SCALE, scale=SCALE
                )
                t1q = wp.tile([P, 2 * CB], F32)
                nc.scalar.activation(t1q[:], xq_s[:], AF.Copy, bias=MAGIC, scale=1.0)
                gq = wp.tile([P, 2 * CB], F32)
                nc.vector.scalar_tensor_tensor(
                    gq[:], t1q[:], -MAGIC, xq_s[:], OP.add, OP.is_gt
                )
                # lo = (t1 - MAGIC) - g  (exact floor), in place over t1q
                nc.vector.scalar_tensor_tensor(
                    t1q[:], t1q[:], -MAGIC, gq[:], OP.add, OP.subtract
                )
                # idx = lo_hi*128 + lo_lo, int16 cast fused into the output
                lo_pair = t1q[:].rearrange("p (m two) -> p m two", two=2)
                stage = wp.tile([P, CB], I16)
                nc.vector.scalar_tensor_tensor(
                    stage[:],
                    lo_pair[:, :, 0],
                    128.0,
                    lo_pair[:, :, 1],
                    OP.mult,
                    OP.add,
                )
                # marshal row-blocks into LL rows 0-15 (spread across engine
                # DMA queues), then replicate to rows 16-31.
                LLf = LL[:].rearrange("p a b c -> p (a b c)")
                engs = (nc.sync, nc.scalar)
                for rb in range(NB):
                    engs[rb % 2].dma_start(
                        LLf[0:16, 2 * LC * 2 * rb : 2 * LC * 2 * (rb + 1)],
                        stage[16 * rb : 16 * rb + 16, :],
                    )
                nc.sync.dma_start(LLf[16:32, :], LLf[0:16, :])

                # ---------------- interp weights ----------------
                # x_s is [128, (256 j, 4 d)]; w = frac(xc), a = 1 - w, stored
                # planar f16 in aw = [128, (a|w), 4 d, 256 j] so downstream
                # ops keep stride-1 innermost dims (DVE 2x/4x perf modes).
                x_s = wp.tile([P, J * 4], F32)
                nc.scalar.dma_start(x_s[:], x_pm[:].rearrange("p a b -> p (a b)"))
                nc.scalar.activation(x_s[:], x_s[:], AF.Copy, bias=SCALE, scale=SCALE)
                t1 = wp.tile([P, J * 4], F32)
                nc.scalar.activation(t1[:], x_s[:], AF.Copy, bias=MAGIC, scale=1.0)
                gw = wp.tile([P, J * 4], F32)
                nc.vector.scalar_tensor_tensor(
                    gw[:], t1[:], -MAGIC, x_s[:], OP.add, OP.is_gt
                )
                # s1 = (t1 - MAGIC) - xc = t - xc   (exact: t1 - MAGIC is exact)
                s1 = wp.tile([P, J * 4], F32)
                nc.vector.scalar_tensor_tensor(
                    s1[:], t1[:], -MAGIC, x_s[:], OP.add, OP.subtract
                )
                aw = wp.tile([P, 2, 4, J], F16)
                # w = g - (t - xc) = xc - floor(xc); write planar-transposed
                gv = gw[:].rearrange("p (j d) -> p d j", d=4)
                sv = s1[:].rearrange("p (j d) -> p d j", d=4)
                nc.vector.tensor_tensor(aw[:, 1], gv, sv, OP.subtract)
                # a = 1 - w (f16 all through, 4x-eligible)
                nc.vector.tensor_scalar(
                    aw[:, 0], aw[:, 1], -1.0, 1.0, OP.mult, OP.add
                )
                # corner products: c order = (hi,lo) in {(a,a),(a,w),(w,a),(w,w)}
                # G uses dims (0 hi, 1 lo); H uses dims (3 hi, 2 lo).
                for W, dhi, dlo in ((Wg, 0, 1), (Wh, 3, 2)):
                    in1 = aw[:, :, dlo, :]  # [P, (a|w), J]
                    for ci, sel in ((0, 0), (2, 1)):
                        in0 = aw[:, sel, dhi, :].unsqueeze(1).broadcast_to(
                            [P, 2, J]
                        )
                        nc.vector.tensor_tensor(
                            W[:, ci : ci + 2, :], in0, in1, OP.mult
                        )

            # ---------------- gather + combine ----------------
            with (
                tc.tile_pool(name="gbuf", bufs=4) as gb,
                tc.tile_pool(name="cbuf", bufs=2) as cb,
            ):
                for ch in range(NCH):
                    gG = gb.tile([P, JC, ES], F16, tag="gG")
                    nc.gpsimd.dma_gather(
                        gG[:],
                        tabG,
                        LL[:, ch, 0, :],
                        NIDX,
                        NIDX,
                        ES,
                        queue_num=0,
                        single_packet=False,
                    )
                    gH = gb.tile([P, JC, ES], F16, tag="gH")
                    nc.gpsimd.dma_gather(
                        gH[:],
                        tabH,
                        LL[:, ch, 1, :],
                        NIDX,
                        NIDX,
                        ES,
                        queue_num=0,
                        single_packet=False,
                    )

                    uv = []
                    for ti, (g, W) in enumerate(((gG, Wg), (gH, Wh))):
                        # m[c, j, k] = gathered corner value * corner weight
                        # (weight broadcast over k via stride-0 AP)
                        m = cb.tile([P, 4, JC, R], F16, tag=f"m{ti}")
                        gv2 = g[:, :, 0 : 4 * R].rearrange(
                            "p j (c k) -> p c j k", k=R
                        )
                        wbc = (
                            W[:, :, JC * ch : JC * ch + JC]
                            .unsqueeze(3)
                            .broadcast_to([P, 4, JC, R])
                        )
                        nc.vector.tensor_tensor(m[:], gv2, wbc, OP.mult)
                        t2 = cb.tile([P, 2, JC, R], F16, tag=f"t{ti}")
                        nc.vector.tensor_tensor(
                            t2[:], m[:, 0:2], m[:, 2:4], OP.add
                        )
                        u = cb.tile([P, JC, R], F16, tag=f"u{ti}")
                        nc.vector.tensor_tensor(u[:], t2[:, 0], t2[:, 1], OP.add)
                        uv.append(u)

                    pr = cb.tile([P, JC, R], F16, tag="pr")
                    nc.vector.tensor_tensor(pr[:], uv[0][:], uv[1][:], OP.mult)
                    nc.vector.tensor_reduce(
                        ysb[:, JC * ch : JC * ch + JC],
                        pr[:],
                        mybir.AxisListType.X,
                        OP.add,
                    )

            nc.sync.dma_start(y_pm[:], ysb[:])

    nc.finalize()
    return nc


def _pack_table(T):
    """T [128, 128, 16] f32 -> [16384, 128] f16, 4-corner packed + pad."""
    ar = np.arange(N)
    out = np.zeros((N, N, ES), dtype=np.float16)
    for ci, (dn, dm) in enumerate(((0, 0), (0, 1), (1, 0), (1, 1))):
        rn = np.minimum(ar + dn, N - 1)
        rm = np.minimum(ar + dm, N - 1)
        out[:, :, ci * R : ci * R + R] = T[np.ix_(rn, rm)].astype(np.float16)
    return np.ascontiguousarray(out.reshape(TE, ES))


def _prep_inputs(x, core0, core1, core2, core3):
    """Host-side input marshalling: shard x over cores, lay out tensors in
    the on-chip layouts the kernel expects, and precompute the 4-corner
    packed f16 pair tables (weight preprocessing, O(N^2 R^2))."""
    xs = np.ascontiguousarray(np.asarray(x, dtype=np.float32).reshape(NCORES, BS, 4))

    core0 = np.asarray(core0, dtype=np.float32)
    core1 = np.asarray(core1, dtype=np.float32)
    core2 = np.asarray(core2, dtype=np.float32)
    core3 = np.asarray(core3, dtype=np.float32)

    # G[n0, n1, k] = sum_c core0[0, n0, c] core1[c, n1, k]
    G = np.einsum("nc,cmk->nmk", core0[0], core1, optimize=True)
    # H[n3, n2, k] = sum_c core3[c, n3, 0] core2[k, n2, c]
    H = np.einsum("cn,kmc->nmk", core3[:, :, 0], core2, optimize=True)
    tab = np.concatenate([_pack_table(G), _pack_table(H)], axis=0)

    in_maps = []
    for c in range(NCORES):
        xc_ = xs[c]
        x_pm = np.ascontiguousarray(
            xc_.reshape(J, P, 4).transpose(1, 0, 2)
        )  # [128, 256, 4]
        # coordinate pairs for the index lists: point i of chunk ch sits at
        # wrapped list position [i%16, i//16]; row-block rb = ch//2 covers
        # cols [G ch | H ch] for ch = 2rb, 2rb+1.
        xg = xc_[:, [0, 1]].reshape(NCH, LC, 16, 2).transpose(0, 2, 1, 3)
        xh = xc_[:, [3, 2]].reshape(NCH, LC, 16, 2).transpose(0, 2, 1, 3)
        # per chunk: [16, LC, 2] G and H blocks side by side -> [16, 2LC, 2]
        chs = np.concatenate([xg, xh], axis=2)  # [NCH, 16, 2LC, 2]
        # row-block rb stacks chunks 2rb, 2rb+1 along cols -> [16, 4LC, 2]
        xqa = chs.reshape(NB, 2, 16, 2 * LC, 2).transpose(0, 2, 1, 3, 4)
        xqa = xqa.reshape(NB * 16, CB, 2)  # [128, 256, 2]
        in_maps.append(
            {
                "x_pm": x_pm,
                "xq": np.ascontiguousarray(xqa),
                "tab": tab,
            }
        )
    return in_maps


def kernel(x, core0, core1, core2, core3):
    global _CACHED
    if _CACHED is None:
        _CACHED = _build_nc()
    nc = _CACHED
    in_maps = _prep_inputs(x, core0, core1, core2, core3)
    res = run_bass_kernel_spmd(nc, in_maps, core_ids=list(range(NCORES)))
    outs = []
    for c in range(NCORES):
        y_pm = res.results[c]["y_pm"]          # [128, 256]
        outs.append(np.ascontiguousarray(np.asarray(y_pm).T).reshape(-1))
    return np.concatenate(outs).astype(np.float32)


# revision 9
# speedup vs baseline: 1.2356x; 1.0151x over previous
"""Trainium2 Bass kernel for nn_ModelConTT_46016279609475 (TT interpolation).

y[b] = v0[b]^T V1[b] V2[b] v3[b], where v_i are linearly-interpolated slices
of tiny TT cores at per-point grid coordinates derived from x[b, :].

Strategy (per NeuronCore, data-parallel over B):
  * The joint pair tables are built ON HOST (weight preprocessing, O(N^2 R^2)
    one-time work independent of B):
      G[n0, n1, k] = sum_c core0[n0, c] * core1[c, n1, k]      (u-side)
      H[n3, n2, k] = sum_c core3[c, n3] * core2[k, n2, c]      (v-side)
    packed 4-corner per cell in f16 (64 useful + 64 pad values = 256 B, the
    minimum legal dma_gather element):
      TAB[(n*128+m), (dn*2+dm)*16 + k] = T[n+dn, m+dm, k]
    so one dma_gather element fetches everything needed for the bilinear
    interpolation of u[b] (and same for v[b]).
  * Device: compute per-point cell ids (int16) + corner weights (f16) from x,
    then 2 dma_gathers per chunk (one per table) and a DVE combine:
      y[b] = sum_k (sum_c wG_c gG[c,k]) * (sum_c wH_c gH[c,k])

Batch mapping per core: shard b of size 32768; on-chip layout is "p-minor":
element i lives at partition i%128, free col i//128, matching dma_gather's
output layout dst[i%128, i//128]. Index lists are mod-16 wrapped as
dma_gather requires (idx for i at [i%16, i//16]) and replicated to rows
16-31 (the queue's core pair); rows 32-127 are zeroed on the Pool engine so
the full-height index AP passes validation. The index math runs on a
[128, 512]-shaped coordinate tile (8 row-blocks of 16 rows, one per pair of
chunks) so every partition carries real data; small per-block DMAs then
marshal the int16 lists into the wrapped layout.

Exact-floor trick (f32-safe): t = (xc + 2^23) - 2^23 rounds to nearest;
g = (t > xc); floor = t - g; frac = xc - floor computed via the exact
(t1 - 2^23) path to avoid re-rounding.
"""

import numpy as np

import concourse.bass as bass
import concourse.bacc as bacc
import concourse.mybir as mybir
import concourse.tile as tile
from concourse import library_config
from concourse.bass_utils import run_bass_kernel_spmd

F32 = mybir.dt.float32
F16 = mybir.dt.float16
I16 = mybir.dt.int16
OP = mybir.AluOpType
AF = mybir.ActivationFunctionType

NCORES = 8
B = 262144
BS = B // NCORES          # 32768 points per core
P = 128                   # partitions
J = BS // P               # 256 free cols per partition
NCH = 16                  # pipeline chunks
JC = J // NCH             # 16 cols per chunk
NIDX = P * JC             # 2048 idxs per gather
LC = NIDX // 16           # 128 idx-list cols per chunk per table
N = 128                   # mode size
R = 16                    # TT rank
TE = N * N                # table entries per side
ES = 128                  # f16 elems per gather element (64 useful + 64 pad)
MAGIC = float(2 ** 23)
SCALE = (N - 1) / 2.0     # 63.5
NB = 8                    # row-blocks in the coordinate tile
CB = 2 * BS // 16 // NB   # 512 pair-cols per row (G+H, 2 chunks)

_CACHED = None


def _build_nc():
    nc = bacc.Bacc("TRN2")

    x_pm = nc.dram_tensor("x_pm", [P, J, 4], F32, kind="ExternalInput")
    xq = nc.dram_tensor("xq", [P, CB, 2], F32, kind="ExternalInput")
    xq0 = nc.dram_tensor("xq0", [32, CB, 2], F32, kind="ExternalInput")
    tab = nc.dram_tensor("tab", [2 * TE, ES], F16, kind="ExternalInput")
    y_pm = nc.dram_tensor("y_pm", [P, J], F32, kind="ExternalOutput")

    tabG = tab[0:TE, :]
    tabH = tab[TE : 2 * TE, :]

    with tile.TileContext(nc) as tc:
        with tc.tile_pool(name="per", bufs=1) as pe:
            nc.gpsimd.load_library(library_config.mlp)

            # persistent tiles. LL holds both tables' index lists, laid out
            # [128, ch, side, LC]: per chunk a G block then an H block, each
            # a contiguous LC-col run (so each gather's idx AP is contiguous).
            LL = pe.tile([P, NCH, 2, LC], I16)
            nc.gpsimd.memset(LL[:], 0)
            Wg = pe.tile([P, 4, J], F16)
            Wh = pe.tile([P, 4, J], F16)
            ysb = pe.tile([P, J], F32)

            with tc.tile_pool(name="pre", bufs=1) as wp:
                LLf = LL[:].rearrange("p a b c -> p (a b c)")
                engs = (nc.sync, nc.scalar)

                # -------- mini index chain: chunks 0-1 straight into LL ----
                # xq0 rows 0-15 hold chunk 0-1 coordinate pairs in wrapped
                # list order; rows 16-31 are a host-made duplicate, so one
                # int16 store covers LL rows 0-31 with no marshal/replica DMA
                # and the first gathers can launch while the main chain runs.
                xq0_s = wp.tile([32, 2 * CB], F32)
                nc.sync.dma_start(xq0_s[:], xq0[:].rearrange("p a b -> p (a b)"))
                nc.scalar.activation(
                    xq0_s[:], xq0_s[:], AF.Copy, bias=SCALE, scale=SCALE
                )
                t0q = wp.tile([32, 2 * CB], F32)
                nc.scalar.activation(t0q[:], xq0_s[:], AF.Copy, bias=MAGIC, scale=1.0)
                g0q = wp.tile([32, 2 * CB], F32)
                nc.vector.scalar_tensor_tensor(
                    g0q[:], t0q[:], -MAGIC, xq0_s[:], OP.add, OP.is_gt
                )
                nc.vector.scalar_tensor_tensor(
                    t0q[:], t0q[:], -MAGIC, g0q[:], OP.add, OP.subtract
                )
                lo0_pair = t0q[:].rearrange("p (m two) -> p m two", two=2)
                nc.vector.scalar_tensor_tensor(
                    LLf[0:32, 0:CB],
                    lo0_pair[:, :, 0],
                    128.0,
                    lo0_pair[:, :, 1],
                    OP.mult,
                    OP.add,
                )

                # -------- main index chain: chunks 2-15 ----------
                # xq holds (hi, lo) coordinate pairs; row-block rb (16 rows)
                # covers chunks 2rb..2rb+1 worth of list columns in wrapped
                # order: [G ch | H ch | G ch+1 | H ch+1] per block.
                xq_s = wp.tile([P, 2 * CB], F32)
                nc.sync.dma_start(xq_s[:], xq[:].rearrange("p a b -> p (a b)"))
                nc.scalar.activation(
                    xq_s[:], xq_s[:], AF.Copy, bias=SCALE, scale=SCALE
                )
                t1q = wp.tile([P, 2 * CB], F32)
                nc.scalar.activation(t1q[:], xq_s[:], AF.Copy, bias=MAGIC, scale=1.0)
                gq = wp.tile([P, 2 * CB], F32)
                nc.vector.scalar_tensor_tensor(
                    gq[:], t1q[:], -MAGIC, xq_s[:], OP.add, OP.is_gt
                )
                # lo = (t1 - MAGIC) - g  (exact floor), in place over t1q
                nc.vector.scalar_tensor_tensor(
                    t1q[:], t1q[:], -MAGIC, gq[:], OP.add, OP.subtract
                )
                # idx = lo_hi*128 + lo_lo, int16 cast fused into the output
                lo_pair = t1q[:].rearrange("p (m two) -> p m two", two=2)
                stage = wp.tile([P, CB], I16)
                nc.vector.scalar_tensor_tensor(
                    stage[:],
                    lo_pair[:, :, 0],
                    128.0,
                    lo_pair[:, :, 1],
                    OP.mult,
                    OP.add,
                )
                # marshal row-blocks 1-7 (chunks 2-15) into LL rows 0-15 and
                # replicate each to rows 16-31, earliest chunks first, DMAs
                # spread across the SP and Act queues.
                for i, rb in enumerate(range(1, NB)):
                    cols = slice(CB * rb, CB * (rb + 1))
                    engs[i % 2].dma_start(
                        LLf[0:16, cols], stage[16 * rb : 16 * rb + 16, :]
                    )
                    engs[(i + 1) % 2].dma_start(LLf[16:32, cols], LLf[0:16, cols])

                # ---------------- interp weights ----------------
                # x_s is [128, (256 j, 4 d)]; w = frac(xc), a = 1 - w, stored
                # planar f16 in aw = [128, (a|w), 4 d, 256 j] so downstream
                # ops keep stride-1 innermost dims (DVE 2x/4x perf modes).
                x_s = wp.tile([P, J * 4], F32)
                nc.scalar.dma_start(x_s[:], x_pm[:].rearrange("p a b -> p (a b)"))
                nc.scalar.activation(x_s[:], x_s[:], AF.Copy, bias=SCALE, scale=SCALE)
                t1 = wp.tile([P, J * 4], F32)
                nc.scalar.activation(t1[:], x_s[:], AF.Copy, bias=MAGIC, scale=1.0)
                gw = wp.tile([P, J * 4], F32)
                nc.vector.scalar_tensor_tensor(
                    gw[:], t1[:], -MAGIC, x_s[:], OP.add, OP.is_gt
                )
                # s1 = (t1 - MAGIC) - xc = t - xc   (exact: t1 - MAGIC is exact)
                s1 = wp.tile([P, J * 4], F32)
                nc.vector.scalar_tensor_tensor(
                    s1[:], t1[:], -MAGIC, x_s[:], OP.add, OP.subtract
                )
                aw = wp.tile([P, 2, 4, J], F16)
                # w = g - (t - xc) = xc - floor(xc); write planar-transposed
                gv = gw[:].rearrange("p (j d) -> p d j", d=4)
                sv = s1[:].rearrange("p (j d) -> p d j", d=4)
                nc.vector.tensor_tensor(aw[:, 1], gv, sv, OP.subtract)
                # a = 1 - w (f16 all through, 4x-eligible)
                nc.vector.tensor_scalar(
                    aw[:, 0], aw[:, 1], -1.0, 1.0, OP.mult, OP.add
                )
                # corner products: c order = (hi,lo) in {(a,a),(a,w),(w,a),(w,w)}
                # G uses dims (0 hi, 1 lo); H uses dims (3 hi, 2 lo).
                for W, dhi, dlo in ((Wg, 0, 1), (Wh, 3, 2)):
                    in1 = aw[:, :, dlo, :]  # [P, (a|w), J]
                    for ci, sel in ((0, 0), (2, 1)):
                        in0 = aw[:, sel, dhi, :].unsqueeze(1).broadcast_to(
                            [P, 2, J]
                        )
                        nc.vector.tensor_tensor(
                            W[:, ci : ci + 2, :], in0, in1, OP.mult
                        )

            # ---------------- gather + combine ----------------
            with (
                tc.tile_pool(name="gbuf", bufs=4) as gb,
                tc.tile_pool(name="cbuf", bufs=2) as cb,
            ):
                for ch in range(NCH):
                    gG = gb.tile([P, JC, ES], F16, tag="gG")
                    nc.gpsimd.dma_gather(
                        gG[:],
                        tabG,
                        LL[:, ch, 0, :],
                        NIDX,
                        NIDX,
                        ES,
                        queue_num=0,
                        single_packet=False,
                    )
                    gH = gb.tile([P, JC, ES], F16, tag="gH")
                    nc.gpsimd.dma_gather(
                        gH[:],
                        tabH,
                        LL[:, ch, 1, :],
                        NIDX,
                        NIDX,
                        ES,
                        queue_num=0,
                        single_packet=False,
                    )

                    uv = []
                    for ti, (g, W) in enumerate(((gG, Wg), (gH, Wh))):
                        # m[c, j, k] = gathered corner value * corner weight
                        # (weight broadcast over k via stride-0 AP)
                        m = cb.tile([P, 4, JC, R], F16, tag=f"m{ti}")
                        gv2 = g[:, :, 0 : 4 * R].rearrange(
                            "p j (c k) -> p c j k", k=R
                        )
                        wbc = (
                            W[:, :, JC * ch : JC * ch + JC]
                            .unsqueeze(3)
                            .broadcast_to([P, 4, JC, R])
                        )
                        nc.vector.tensor_tensor(m[:], gv2, wbc, OP.mult)
                        t2 = cb.tile([P, 2, JC, R], F16, tag=f"t{ti}")
                        nc.vector.tensor_tensor(
                            t2[:], m[:, 0:2], m[:, 2:4], OP.add
                        )
                        u = cb.tile([P, JC, R], F16, tag=f"u{ti}")
                        nc.vector.tensor_tensor(u[:], t2[:, 0], t2[:, 1], OP.add)
                        uv.append(u)

                    pr = cb.tile([P, JC, R], F16, tag="pr")
                    nc.vector.tensor_tensor(pr[:], uv[0][:], uv[1][:], OP.mult)
                    nc.vector.tensor_reduce(
                        ysb[:, JC * ch : JC * ch + JC],
                        pr[:],
                        mybir.AxisListType.X,
                        OP.add,
                    )
                    # stream the output back in halves so only the last
                    # sliver of writeback sits after the final reduce
                    if ch == NCH // 2 - 1:
                        nc.sync.dma_start(
                            y_pm[:, 0 : J // 2], ysb[:, 0 : J // 2]
                        )

            nc.sync.dma_start(y_pm[:, J // 2 : J], ysb[:, J // 2 : J])

    nc.finalize()
    return nc


def _pack_table(T):
    """T [128, 128, 16] f32 -> [16384, 128] f16, 4-corner packed + pad."""
    ar = np.arange(N)
    out = np.zeros((N, N, ES), dtype=np.float16)
    for ci, (dn, dm) in enumerate(((0, 0), (0, 1), (1, 0), (1, 1))):
        rn = np.minimum(ar + dn, N - 1)
        rm = np.minimum(ar + dm, N - 1)
        out[:, :, ci * R : ci * R + R] = T[np.ix_(rn, rm)].astype(np.float16)
    return np.ascontiguousarray(out.reshape(TE, ES))


def _prep_inputs(x, core0, core1, core2, core3):
    """Host-side input marshalling: shard x over cores, lay out tensors in
    the on-chip layouts the kernel expects, and precompute the 4-corner
    packed f16 pair tables (weight preprocessing, O(N^2 R^2))."""
    xs = np.ascontiguousarray(np.asarray(x, dtype=np.float32).reshape(NCORES, BS, 4))

    core0 = np.asarray(core0, dtype=np.float32)
    core1 = np.asarray(core1, dtype=np.float32)
    core2 = np.asarray(core2, dtype=np.float32)
    core3 = np.asarray(core3, dtype=np.float32)

    # G[n0, n1, k] = sum_c core0[0, n0, c] core1[c, n1, k]
    G = np.einsum("nc,cmk->nmk", core0[0], core1, optimize=True)
    # H[n3, n2, k] = sum_c core3[c, n3, 0] core2[k, n2, c]
    H = np.einsum("cn,kmc->nmk", core3[:, :, 0], core2, optimize=True)
    tab = np.concatenate([_pack_table(G), _pack_table(H)], axis=0)

    in_maps = []
    for c in range(NCORES):
        xc_ = xs[c]
        x_pm = np.ascontiguousarray(
            xc_.reshape(J, P, 4).transpose(1, 0, 2)
        )  # [128, 256, 4]
        # coordinate pairs for the index lists: point i of chunk ch sits at
        # wrapped list position [i%16, i//16]; row-block rb = ch//2 covers
        # cols [G ch | H ch] for ch = 2rb, 2rb+1.
        xg = xc_[:, [0, 1]].reshape(NCH, LC, 16, 2).transpose(0, 2, 1, 3)
        xh = xc_[:, [3, 2]].reshape(NCH, LC, 16, 2).transpose(0, 2, 1, 3)
        # per chunk: [16, LC, 2] G and H blocks side by side -> [16, 2LC, 2]
        chs = np.concatenate([xg, xh], axis=2)  # [NCH, 16, 2LC, 2]
        # row-block rb stacks chunks 2rb, 2rb+1 along cols -> [16, 4LC, 2]
        xqa = chs.reshape(NB, 2, 16, 2 * LC, 2).transpose(0, 2, 1, 3, 4)
        xqa = xqa.reshape(NB * 16, CB, 2)  # [128, 512, 2]
        # mini tile: chunks 0-1 in rows 0-15, duplicated into rows 16-31
        xq0 = np.concatenate([xqa[0:16], xqa[0:16]], axis=0)  # [32, 512, 2]
        in_maps.append(
            {
                "x_pm": x_pm,
                "xq": np.ascontiguousarray(xqa),
                "xq0": np.ascontiguousarray(xq0),
                "tab": tab,
            }
        )
    return in_maps


def kernel(x, core0, core1, core2, core3):
    global _CACHED
    if _CACHED is None:
        _CACHED = _build_nc()
    nc = _CACHED
    in_maps = _prep_inputs(x, core0, core1, core2, core3)
    res = run_bass_kernel_spmd(nc, in_maps, core_ids=list(range(NCORES)))
    outs = []
    for c in range(NCORES):
        y_pm = res.results[c]["y_pm"]          # [128, 256]
        outs.append(np.ascontiguousarray(np.asarray(y_pm).T).reshape(-1))
    return np.concatenate(outs).astype(np.float32)


# revision 12
# speedup vs baseline: 1.3012x; 1.0531x over previous
"""Trainium2 Bass kernel for nn_ModelConTT_46016279609475 (TT interpolation).

y[b] = v0[b]^T V1[b] V2[b] v3[b], where v_i are linearly-interpolated slices
of tiny TT cores at per-point grid coordinates derived from x[b, :].

Strategy (per NeuronCore, data-parallel over B):
  * The joint pair tables are built ON HOST (weight preprocessing, O(N^2 R^2)
    one-time work independent of B):
      G[n0, n1, k] = sum_c core0[n0, c] * core1[c, n1, k]      (u-side)
      H[n3, n2, k] = sum_c core3[c, n3] * core2[k, n2, c]      (v-side)
    packed 4-corner per cell in f16 (64 useful + 64 pad values = 256 B, the
    minimum legal dma_gather element):
      TAB[(n*128+m), (dn*2+dm)*16 + k] = T[n+dn, m+dm, k]
    so one dma_gather element fetches everything needed for the bilinear
    interpolation of u[b] (and same for v[b]).
  * Device: compute per-point cell ids (int16) + corner weights (f16) from x,
    then 2 dma_gathers per chunk (one per table) and a DVE combine:
      y[b] = sum_k (sum_c wG_c gG[c,k]) * (sum_c wH_c gH[c,k])

Batch mapping per core: shard b of size 32768; on-chip layout is "p-minor":
element i lives at partition i%128, free col i//128, matching dma_gather's
output layout dst[i%128, i//128]. Index lists are mod-16 wrapped as
dma_gather requires (idx for i at [i%16, i//16]) and replicated to rows
16-31 (the queue's core pair); rows 32-127 are zeroed on the Pool engine so
the full-height index AP passes validation. The index math runs on a
[128, 512]-shaped coordinate tile (8 row-blocks of 16 rows, one per pair of
chunks) so every partition carries real data; small per-block DMAs then
marshal the int16 lists into the wrapped layout.

Exact-floor trick (f32-safe): t = (xc + 2^23) - 2^23 rounds to nearest;
g = (t > xc); floor = t - g; frac = xc - floor computed via the exact
(t1 - 2^23) path to avoid re-rounding.
"""

import numpy as np

import concourse.bass as bass
import concourse.bacc as bacc
import concourse.mybir as mybir
import concourse.tile as tile
from concourse import library_config
from concourse.bass_utils import run_bass_kernel_spmd

F32 = mybir.dt.float32
F16 = mybir.dt.float16
I16 = mybir.dt.int16
OP = mybir.AluOpType
AF = mybir.ActivationFunctionType

NCORES = 8
B = 262144
BS = B // NCORES          # 32768 points per core
P = 128                   # partitions
J = BS // P               # 256 free cols per partition
NCH = 16                  # pipeline chunks
JC = J // NCH             # 16 cols per chunk
NIDX = P * JC             # 2048 idxs per gather
LC = NIDX // 16           # 128 idx-list cols per chunk per table
N = 128                   # mode size
R = 16                    # TT rank
TE = N * N                # table entries per side
ES = 128                  # f16 elems per gather element (64 useful + 64 pad)
MAGIC = float(2 ** 23)
SCALE = (N - 1) / 2.0     # 63.5
NB = 8                    # row-blocks in the coordinate tile
CB = 2 * BS // 16 // NB   # 512 pair-cols per row (G+H, 2 chunks)

_CACHED = None


def _build_nc():
    nc = bacc.Bacc("TRN2")

    x_pm = nc.dram_tensor("x_pm", [P, J, 4], F32, kind="ExternalInput")
    xq = nc.dram_tensor("xq", [P, CB, 2], F32, kind="ExternalInput")
    xq0 = nc.dram_tensor("xq0", [32, CB, 2], F32, kind="ExternalInput")
    tab = nc.dram_tensor("tab", [2 * TE, ES], F16, kind="ExternalInput")
    y_pm = nc.dram_tensor("y_pm", [P, J], F32, kind="ExternalOutput")

    tabG = tab[0:TE, :]
    tabH = tab[TE : 2 * TE, :]

    with tile.TileContext(nc) as tc:
        # all pools stay open for the whole kernel: a pool close emits a
        # release barrier that would serialize the gather loop behind the
        # entire preamble (SBUF is plentiful here, ~105 KB/partition peak)
        with (
            tc.tile_pool(name="per", bufs=1) as pe,
            tc.tile_pool(name="wp", bufs=1) as wp,
            tc.tile_pool(name="gbuf", bufs=4) as gb,
            tc.tile_pool(name="cbuf", bufs=2) as cb,
        ):
            nc.gpsimd.load_library(library_config.mlp)

            # persistent tiles. LL holds both tables' index lists, laid out
            # [128, ch, side, LC]: per chunk a G block then an H block, each
            # a contiguous LC-col run (so each gather's idx AP is contiguous).
            LL = pe.tile([P, NCH, 2, LC], I16)
            nc.gpsimd.memset(LL[:], 0)
            Wg = pe.tile([P, 4, J], F16)
            Wh = pe.tile([P, 4, J], F16)
            ysb = pe.tile([P, J], F32)

            if True:
                LLf = LL[:].rearrange("p a b c -> p (a b c)")
                engs = (nc.sync, nc.scalar)

                # -------- mini index chain: chunks 0-1 straight into LL ----
                # xq0 rows 0-15 hold chunk 0-1 coordinate pairs in wrapped
                # list order; rows 16-31 are a host-made duplicate, so one
                # int16 store covers LL rows 0-31 with no marshal/replica DMA
                # and the first gathers can launch while the main chain runs.
                xq0_s = wp.tile([32, 2 * CB], F32)
                nc.sync.dma_start(xq0_s[:], xq0[:].rearrange("p a b -> p (a b)"))
                nc.scalar.activation(
                    xq0_s[:], xq0_s[:], AF.Copy, bias=SCALE, scale=SCALE
                )
                t0q = wp.tile([32, 2 * CB], F32)
                nc.scalar.activation(t0q[:], xq0_s[:], AF.Copy, bias=MAGIC, scale=1.0)
                g0q = wp.tile([32, 2 * CB], F32)
                nc.vector.scalar_tensor_tensor(
                    g0q[:], t0q[:], -MAGIC, xq0_s[:], OP.add, OP.is_gt
                )
                nc.vector.scalar_tensor_tensor(
                    t0q[:], t0q[:], -MAGIC, g0q[:], OP.add, OP.subtract
                )
                lo0_pair = t0q[:].rearrange("p (m two) -> p m two", two=2)
                nc.vector.scalar_tensor_tensor(
                    LLf[0:32, 0:CB],
                    lo0_pair[:, :, 0],
                    128.0,
                    lo0_pair[:, :, 1],
                    OP.mult,
                    OP.add,
                )

                # -------- main index chain: chunks 2-15 ----------
                # xq holds (hi, lo) coordinate pairs; row-block rb (16 rows)
                # covers chunks 2rb..2rb+1 worth of list columns in wrapped
                # order: [G ch | H ch | G ch+1 | H ch+1] per block.
                xq_s = wp.tile([P, 2 * CB], F32)
                nc.sync.dma_start(xq_s[:], xq[:].rearrange("p a b -> p (a b)"))
                nc.scalar.activation(
                    xq_s[:], xq_s[:], AF.Copy, bias=SCALE, scale=SCALE
                )
                t1q = wp.tile([P, 2 * CB], F32)
                nc.scalar.activation(t1q[:], xq_s[:], AF.Copy, bias=MAGIC, scale=1.0)
                gq = wp.tile([P, 2 * CB], F32)
                nc.vector.scalar_tensor_tensor(
                    gq[:], t1q[:], -MAGIC, xq_s[:], OP.add, OP.is_gt
                )
                # lo = (t1 - MAGIC) - g  (exact floor), in place over t1q
                nc.vector.scalar_tensor_tensor(
                    t1q[:], t1q[:], -MAGIC, gq[:], OP.add, OP.subtract
                )
                # idx = lo_hi*128 + lo_lo, int16 cast fused into the output
                lo_pair = t1q[:].rearrange("p (m two) -> p m two", two=2)
                stage = wp.tile([P, CB], I16)
                nc.vector.scalar_tensor_tensor(
                    stage[:],
                    lo_pair[:, :, 0],
                    128.0,
                    lo_pair[:, :, 1],
                    OP.mult,
                    OP.add,
                )
                # marshal row-blocks 1-7 (chunks 2-15) into LL rows 0-15 and
                # replicate each to rows 16-31, earliest chunks first, DMAs
                # spread across the SP and Act queues.
                for i, rb in enumerate(range(1, NB)):
                    cols = slice(CB * rb, CB * (rb + 1))
                    engs[i % 2].dma_start(
                        LLf[0:16, cols], stage[16 * rb : 16 * rb + 16, :]
                    )
                    engs[(i + 1) % 2].dma_start(LLf[16:32, cols], LLf[0:16, cols])

                # ---------------- interp weights ----------------
                # x_s is [128, (256 j, 4 d)]; w = frac(xc), a = 1 - w, stored
                # planar f16 in aw = [128, (a|w), 4 d, 256 j] so downstream
                # ops keep stride-1 innermost dims (DVE 2x/4x perf modes).
                x_s = wp.tile([P, J * 4], F32)
                nc.scalar.dma_start(x_s[:], x_pm[:].rearrange("p a b -> p (a b)"))
                nc.scalar.activation(x_s[:], x_s[:], AF.Copy, bias=SCALE, scale=SCALE)
                t1 = wp.tile([P, J * 4], F32)
                nc.scalar.activation(t1[:], x_s[:], AF.Copy, bias=MAGIC, scale=1.0)
                gw = wp.tile([P, J * 4], F32)
                nc.vector.scalar_tensor_tensor(
                    gw[:], t1[:], -MAGIC, x_s[:], OP.add, OP.is_gt
                )
                # s1 = (t1 - MAGIC) - xc = t - xc   (exact: t1 - MAGIC is exact)
                s1 = wp.tile([P, J * 4], F32)
                nc.vector.scalar_tensor_tensor(
                    s1[:], t1[:], -MAGIC, x_s[:], OP.add, OP.subtract
                )
                aw = wp.tile([P, 2, 4, J], F16)
                # w = g - (t - xc) = xc - floor(xc); write planar-transposed
                gv = gw[:].rearrange("p (j d) -> p d j", d=4)
                sv = s1[:].rearrange("p (j d) -> p d j", d=4)
                nc.vector.tensor_tensor(aw[:, 1], gv, sv, OP.subtract)
                # a = 1 - w (f16 all through, 4x-eligible)
                nc.vector.tensor_scalar(
                    aw[:, 0], aw[:, 1], -1.0, 1.0, OP.mult, OP.add
                )
                # corner products: c order = (hi,lo) in {(a,a),(a,w),(w,a),(w,w)}
                # G uses dims (0 hi, 1 lo); H uses dims (3 hi, 2 lo).
                for W, dhi, dlo in ((Wg, 0, 1), (Wh, 3, 2)):
                    in1 = aw[:, :, dlo, :]  # [P, (a|w), J]
                    for ci, sel in ((0, 0), (2, 1)):
                        in0 = aw[:, sel, dhi, :].unsqueeze(1).broadcast_to(
                            [P, 2, J]
                        )
                        nc.vector.tensor_tensor(
                            W[:, ci : ci + 2, :], in0, in1, OP.mult
                        )

            # ---------------- gather + combine ----------------
            if True:
                for ch in range(NCH):
                    gG = gb.tile([P, JC, ES], F16, tag="gG")
                    nc.gpsimd.dma_gather(
                        gG[:],
                        tabG,
                        LL[:, ch, 0, :],
                        NIDX,
                        NIDX,
                        ES,
                        queue_num=0,
                        single_packet=False,
                    )
                    gH = gb.tile([P, JC, ES], F16, tag="gH")
                    nc.gpsimd.dma_gather(
                        gH[:],
                        tabH,
                        LL[:, ch, 1, :],
                        NIDX,
                        NIDX,
                        ES,
                        queue_num=0,
                        single_packet=False,
                    )

                    uv = []
                    for ti, (g, W) in enumerate(((gG, Wg), (gH, Wh))):
                        # m[c, j, k] = gathered corner value * corner weight
                        # (weight broadcast over k via stride-0 AP)
                        m = cb.tile([P, 4, JC, R], F16, tag=f"m{ti}")
                        gv2 = g[:, :, 0 : 4 * R].rearrange(
                            "p j (c k) -> p c j k", k=R
                        )
                        wbc = (
                            W[:, :, JC * ch : JC * ch + JC]
                            .unsqueeze(3)
                            .broadcast_to([P, 4, JC, R])
                        )
                        nc.vector.tensor_tensor(m[:], gv2, wbc, OP.mult)
                        t2 = cb.tile([P, 2, JC, R], F16, tag=f"t{ti}")
                        nc.vector.tensor_tensor(
                            t2[:], m[:, 0:2], m[:, 2:4], OP.add
                        )
                        u = cb.tile([P, JC, R], F16, tag=f"u{ti}")
                        nc.vector.tensor_tensor(u[:], t2[:, 0], t2[:, 1], OP.add)
                        uv.append(u)

                    pr = cb.tile([P, JC, R], F16, tag="pr")
                    nc.vector.tensor_tensor(pr[:], uv[0][:], uv[1][:], OP.mult)
                    nc.vector.tensor_reduce(
                        ysb[:, JC * ch : JC * ch + JC],
                        pr[:],
                        mybir.AxisListType.X,
                        OP.add,
                    )
                    # stream the output back in halves so only the last
                    # sliver of writeback sits after the final reduce
                    if ch == NCH // 2 - 1:
                        nc.sync.dma_start(
                            y_pm[:, 0 : J // 2], ysb[:, 0 : J // 2]
                        )

            nc.sync.dma_start(y_pm[:, J // 2 : J], ysb[:, J // 2 : J])

    nc.finalize()
    return nc


def _pack_table(T):
    """T [128, 128, 16] f32 -> [16384, 128] f16, 4-corner packed + pad."""
    ar = np.arange(N)
    out = np.zeros((N, N, ES), dtype=np.float16)
    for ci, (dn, dm) in enumerate(((0, 0), (0, 1), (1, 0), (1, 1))):
        rn = np.minimum(ar + dn, N - 1)
        rm = np.minimum(ar + dm, N - 1)
        out[:, :, ci * R : ci * R + R] = T[np.ix_(rn, rm)].astype(np.float16)
    return np.ascontiguousarray(out.reshape(TE, ES))


def _prep_inputs(x, core0, core1, core2, core3):
    """Host-side input marshalling: shard x over cores, lay out tensors in
    the on-chip layouts the kernel expects, and precompute the 4-corner
    packed f16 pair tables (weight preprocessing, O(N^2 R^2))."""
    xs = np.ascontiguousarray(np.asarray(x, dtype=np.float32).reshape(NCORES, BS, 4))

    core0 = np.asarray(core0, dtype=np.float32)
    core1 = np.asarray(core1, dtype=np.float32)
    core2 = np.asarray(core2, dtype=np.float32)
    core3 = np.asarray(core3, dtype=np.float32)

    # G[n0, n1, k] = sum_c core0[0, n0, c] core1[c, n1, k]
    G = np.einsum("nc,cmk->nmk", core0[0], core1, optimize=True)
    # H[n3, n2, k] = sum_c core3[c, n3, 0] core2[k, n2, c]
    H = np.einsum("cn,kmc->nmk", core3[:, :, 0], core2, optimize=True)
    tab = np.concatenate([_pack_table(G), _pack_table(H)], axis=0)

    in_maps = []
    for c in range(NCORES):
        xc_ = xs[c]
        x_pm = np.ascontiguousarray(
            xc_.reshape(J, P, 4).transpose(1, 0, 2)
        )  # [128, 256, 4]
        # coordinate pairs for the index lists: point i of chunk ch sits at
        # wrapped list position [i%16, i//16]; row-block rb = ch//2 covers
        # cols [G ch | H ch] for ch = 2rb, 2rb+1.
        xg = xc_[:, [0, 1]].reshape(NCH, LC, 16, 2).transpose(0, 2, 1, 3)
        xh = xc_[:, [3, 2]].reshape(NCH, LC, 16, 2).transpose(0, 2, 1, 3)
        # per chunk: [16, LC, 2] G and H blocks side by side -> [16, 2LC, 2]
        chs = np.concatenate([xg, xh], axis=2)  # [NCH, 16, 2LC, 2]
        # row-block rb stacks chunks 2rb, 2rb+1 along cols -> [16, 4LC, 2]
        xqa = chs.reshape(NB, 2, 16, 2 * LC, 2).transpose(0, 2, 1, 3, 4)
        xqa = xqa.reshape(NB * 16, CB, 2)  # [128, 512, 2]
        # mini tile: chunks 0-1 in rows 0-15, duplicated into rows 16-31
        xq0 = np.concatenate([xqa[0:16], xqa[0:16]], axis=0)  # [32, 512, 2]
        in_maps.append(
            {
                "x_pm": x_pm,
                "xq": np.ascontiguousarray(xqa),
                "xq0": np.ascontiguousarray(xq0),
                "tab": tab,
            }
        )
    return in_maps


def kernel(x, core0, core1, core2, core3):
    global _CACHED
    if _CACHED is None:
        _CACHED = _build_nc()
    nc = _CACHED
    in_maps = _prep_inputs(x, core0, core1, core2, core3)
    res = run_bass_kernel_spmd(nc, in_maps, core_ids=list(range(NCORES)))
    outs = []
    for c in range(NCORES):
        y_pm = res.results[c]["y_pm"]          # [128, 256]
        outs.append(np.ascontiguousarray(np.asarray(y_pm).T).reshape(-1))
    return np.concatenate(outs).astype(np.float32)


# revision 14
# speedup vs baseline: 1.3166x; 1.0118x over previous
"""Trainium2 Bass kernel for nn_ModelConTT_46016279609475 (TT interpolation).

y[b] = v0[b]^T V1[b] V2[b] v3[b], where v_i are linearly-interpolated slices
of tiny TT cores at per-point grid coordinates derived from x[b, :].

Strategy (per NeuronCore, data-parallel over B):
  * The joint pair tables are built ON HOST (weight preprocessing, O(N^2 R^2)
    one-time work independent of B):
      G[n0, n1, k] = sum_c core0[n0, c] * core1[c, n1, k]      (u-side)
      H[n3, n2, k] = sum_c core3[c, n3] * core2[k, n2, c]      (v-side)
    packed 4-corner per cell in f16 (64 useful + 64 pad values = 256 B, the
    minimum legal dma_gather element):
      TAB[(n*128+m), (dn*2+dm)*16 + k] = T[n+dn, m+dm, k]
    so one dma_gather element fetches everything needed for the bilinear
    interpolation of u[b] (and same for v[b]).
  * Device: compute per-point cell ids (int16) + corner weights (f16) from x,
    then 2 dma_gathers per chunk (one per table) and a DVE combine:
      y[b] = sum_k (sum_c wG_c gG[c,k]) * (sum_c wH_c gH[c,k])

Batch mapping per core: shard b of size 32768; on-chip layout is "p-minor":
element i lives at partition i%128, free col i//128, matching dma_gather's
output layout dst[i%128, i//128]. Index lists are mod-16 wrapped as
dma_gather requires (idx for i at [i%16, i//16]) and replicated to rows
16-31 (the queue's core pair); rows 32-127 are zeroed on the Pool engine so
the full-height index AP passes validation.

Pipeline startup: chunk sizes are [8,8] + [16]*14 + [8,8] output columns.
A "mini" index chain (DVE) computes chunks 0-2 from a small duplicated-row
coordinate tile straight into LL rows 0-31 (no marshal DMAs), so the first
gathers launch ~12 us in. The main chain for chunks 3-17 runs on the
otherwise-idle GpSimd engine and its lists are marshalled into wrapped
layout by small per-block DMAs during the first chunks' gather runway.
Small first/last chunks shorten pipeline fill and drain.

Exact-floor trick (f32-safe): t = (xc + 2^23) - 2^23 rounds to nearest;
g = (t > xc); floor = t - g; frac = xc - floor computed via the exact
(t1 - 2^23) path to avoid re-rounding.
"""

import numpy as np

import concourse.bass as bass
import concourse.bacc as bacc
import concourse.mybir as mybir
import concourse.tile as tile
from concourse import library_config
from concourse.bass_utils import run_bass_kernel_spmd

F32 = mybir.dt.float32
F16 = mybir.dt.float16
I16 = mybir.dt.int16
OP = mybir.AluOpType
AF = mybir.ActivationFunctionType

NCORES = 8
B = 262144
BS = B // NCORES          # 32768 points per core
P = 128                   # partitions
J = BS // P               # 256 free cols per partition
JCS = [8, 8] + [16] * 14 + [8, 8]   # output cols per chunk (sum = 256)
NCH = len(JCS)
JOFF = np.cumsum([0] + JCS).tolist()
COFF = [16 * o for o in JOFF]       # int16 list-col offset per chunk
N = 128                   # mode size
R = 16                    # TT rank
TE = N * N                # table entries per side
ES = 128                  # f16 elems per gather element (64 useful + 64 pad)
MAGIC = float(2 ** 23)
SCALE = (N - 1) / 2.0     # 63.5
NB = 8                    # row-blocks in the coordinate tile
CB = 2 * BS // 16 // NB   # 512 pair-cols per row (both sides, per block)
LCOLS = 16 * J            # 4096 total int16 list cols
MINI_CH = 3               # chunks covered by the mini chain (= block 0)

_CACHED = None


def _build_nc():
    nc = bacc.Bacc("TRN2")

    x_pm = nc.dram_tensor("x_pm", [P, J, 4], F32, kind="ExternalInput")
    xq = nc.dram_tensor("xq", [P, CB, 2], F32, kind="ExternalInput")
    xq0 = nc.dram_tensor("xq0", [32, CB, 2], F32, kind="ExternalInput")
    tab = nc.dram_tensor("tab", [2 * TE, ES], F16, kind="ExternalInput")
    y_pm = nc.dram_tensor("y_pm", [P, J], F32, kind="ExternalOutput")

    tabG = tab[0:TE, :]
    tabH = tab[TE : 2 * TE, :]

    with tile.TileContext(nc) as tc:
        # all pools stay open for the whole kernel: a pool close emits a
        # release barrier that would serialize the gather loop behind the
        # entire preamble (SBUF is plentiful here, ~105 KB/partition peak)
        with (
            tc.tile_pool(name="per", bufs=1) as pe,
            tc.tile_pool(name="wp", bufs=1) as wp,
            tc.tile_pool(name="gbuf", bufs=4) as gb,
            tc.tile_pool(name="cbuf", bufs=2) as cb,
        ):
            nc.gpsimd.load_library(library_config.mlp)

            # persistent tiles. LL holds both tables' index lists as flat
            # columns; chunk ch occupies cols [COFF[ch], COFF[ch+1]) with a
            # contiguous G block then an H block.
            LL = pe.tile([P, LCOLS], I16)
            nc.gpsimd.memset(LL[:], 0)
            Wg = pe.tile([P, 4, J], F16)
            Wh = pe.tile([P, 4, J], F16)
            ysb = pe.tile([P, J], F32)
            engs = (nc.sync, nc.scalar)

            # -------- mini index chain: chunks 0-2 straight into LL -------
            # xq0 rows 0-15 hold block-0 coordinate pairs in wrapped list
            # order; rows 16-31 are a host-made duplicate, so one int16
            # store covers LL rows 0-31 with no marshal/replica DMA and the
            # first gathers launch while the main chain still runs.
            xq0_s = wp.tile([32, 2 * CB], F32)
            nc.sync.dma_start(xq0_s[:], xq0[:].rearrange("p a b -> p (a b)"))
            nc.scalar.activation(
                xq0_s[:], xq0_s[:], AF.Copy, bias=SCALE, scale=SCALE
            )
            t0q = wp.tile([32, 2 * CB], F32)
            nc.scalar.activation(t0q[:], xq0_s[:], AF.Copy, bias=MAGIC, scale=1.0)
            g0q = wp.tile([32, 2 * CB], F32)
            nc.vector.scalar_tensor_tensor(
                g0q[:], t0q[:], -MAGIC, xq0_s[:], OP.add, OP.is_gt
            )
            nc.vector.scalar_tensor_tensor(
                t0q[:], t0q[:], -MAGIC, g0q[:], OP.add, OP.subtract
            )
            lo0_pair = t0q[:].rearrange("p (m two) -> p m two", two=2)
            nc.vector.scalar_tensor_tensor(
                LL[0:32, 0:CB],
                lo0_pair[:, :, 0],
                128.0,
                lo0_pair[:, :, 1],
                OP.mult,
                OP.add,
            )

            # -------- main index chain (chunks 3-17) on GpSimd ----------
            # keeps the DVE free for the mini chain + weights; GpSimd is
            # otherwise idle until descriptor generation ramps.
            xq_s = wp.tile([P, 2 * CB], F32)
            nc.sync.dma_start(xq_s[:], xq[:].rearrange("p a b -> p (a b)"))
            nc.scalar.activation(
                xq_s[:], xq_s[:], AF.Copy, bias=SCALE, scale=SCALE
            )
            t1q = wp.tile([P, 2 * CB], F32)
            nc.scalar.activation(t1q[:], xq_s[:], AF.Copy, bias=MAGIC, scale=1.0)
            gq = wp.tile([P, 2 * CB], F32)
            nc.vector.scalar_tensor_tensor(
                gq[:], t1q[:], -MAGIC, xq_s[:], OP.add, OP.is_gt
            )
            # lo = (t1 - MAGIC) - g  (exact floor), in place over t1q
            nc.vector.scalar_tensor_tensor(
                t1q[:], t1q[:], -MAGIC, gq[:], OP.add, OP.subtract
            )
            # idx = lo_hi*128 + lo_lo, int16 cast fused into the output
            lo_pair = t1q[:].rearrange("p (m two) -> p m two", two=2)
            stage = wp.tile([P, CB], I16)
            nc.vector.scalar_tensor_tensor(
                stage[:],
                lo_pair[:, :, 0],
                128.0,
                lo_pair[:, :, 1],
                OP.mult,
                OP.add,
            )
            # marshal row-blocks 1-7 (chunks 3-17) into LL rows 0-15 and
            # replicate each to rows 16-31, earliest chunks first, DMAs
            # spread across the SP and Act queues.
            for i, rb in enumerate(range(1, NB)):
                cols = slice(CB * rb, CB * (rb + 1))
                engs[i % 2].dma_start(
                    LL[0:16, cols], stage[16 * rb : 16 * rb + 16, :]
                )
                engs[(i + 1) % 2].dma_start(LL[16:32, cols], LL[0:16, cols])

            # ---------------- interp weights (DVE) ----------------
            # x_s is [128, (256 j, 4 d)]; w = frac(xc), a = 1 - w, stored
            # planar f16 in aw = [128, (a|w), 4 d, 256 j] so downstream
            # ops keep stride-1 innermost dims (DVE 2x/4x perf modes).
            x_s = wp.tile([P, J * 4], F32)
            nc.scalar.dma_start(x_s[:], x_pm[:].rearrange("p a b -> p (a b)"))
            nc.scalar.activation(x_s[:], x_s[:], AF.Copy, bias=SCALE, scale=SCALE)
            t1 = wp.tile([P, J * 4], F32)
            nc.scalar.activation(t1[:], x_s[:], AF.Copy, bias=MAGIC, scale=1.0)
            gw = wp.tile([P, J * 4], F32)
            nc.vector.scalar_tensor_tensor(
                gw[:], t1[:], -MAGIC, x_s[:], OP.add, OP.is_gt
            )
            # s1 = (t1 - MAGIC) - xc = t - xc   (exact: t1 - MAGIC is exact)
            s1 = wp.tile([P, J * 4], F32)
            nc.vector.scalar_tensor_tensor(
                s1[:], t1[:], -MAGIC, x_s[:], OP.add, OP.subtract
            )
            aw = wp.tile([P, 2, 4, J], F16)
            # w = g - (t - xc) = xc - floor(xc); write planar-transposed
            gv = gw[:].rearrange("p (j d) -> p d j", d=4)
            sv = s1[:].rearrange("p (j d) -> p d j", d=4)
            nc.vector.tensor_tensor(aw[:, 1], gv, sv, OP.subtract)
            # a = 1 - w (f16 all through, 4x-eligible)
            nc.vector.tensor_scalar(
                aw[:, 0], aw[:, 1], -1.0, 1.0, OP.mult, OP.add
            )
            # corner products: c order = (hi,lo) in {(a,a),(a,w),(w,a),(w,w)}
            # G uses dims (0 hi, 1 lo); H uses dims (3 hi, 2 lo).
            for W, dhi, dlo in ((Wg, 0, 1), (Wh, 3, 2)):
                in1 = aw[:, :, dlo, :]  # [P, (a|w), J]
                for ci, sel in ((0, 0), (2, 1)):
                    in0 = aw[:, sel, dhi, :].unsqueeze(1).broadcast_to(
                        [P, 2, J]
                    )
                    nc.vector.tensor_tensor(
                        W[:, ci : ci + 2, :], in0, in1, OP.mult
                    )

            # ---------------- gather + combine ----------------
            for ch in range(NCH):
                jc = JCS[ch]
                nidx = P * jc
                gG = gb.tile([P, jc, ES], F16, tag=f"gG{jc}")
                nc.gpsimd.dma_gather(
                    gG[:],
                    tabG,
                    LL[:, COFF[ch] : COFF[ch] + 8 * jc],
                    nidx,
                    nidx,
                    ES,
                    queue_num=0,
                    single_packet=False,
                )
                gH = gb.tile([P, jc, ES], F16, tag=f"gH{jc}")
                nc.gpsimd.dma_gather(
                    gH[:],
                    tabH,
                    LL[:, COFF[ch] + 8 * jc : COFF[ch + 1]],
                    nidx,
                    nidx,
                    ES,
                    queue_num=0,
                    single_packet=False,
                )

                jo = JOFF[ch]
                uv = []
                for ti, (g, W) in enumerate(((gG, Wg), (gH, Wh))):
                    # m[c, j, k] = gathered corner value * corner weight
                    # (weight broadcast over k via stride-0 AP)
                    m = cb.tile([P, 4, jc, R], F16, tag=f"m{ti}{jc}")
                    gv2 = g[:, :, 0 : 4 * R].rearrange(
                        "p j (c k) -> p c j k", k=R
                    )
                    wbc = (
                        W[:, :, jo : jo + jc]
                        .unsqueeze(3)
                        .broadcast_to([P, 4, jc, R])
                    )
                    nc.vector.tensor_tensor(m[:], gv2, wbc, OP.mult)
                    t2 = cb.tile([P, 2, jc, R], F16, tag=f"t{ti}{jc}")
                    nc.vector.tensor_tensor(
                        t2[:], m[:, 0:2], m[:, 2:4], OP.add
                    )
                    u = cb.tile([P, jc, R], F16, tag=f"u{ti}{jc}")
                    nc.vector.tensor_tensor(u[:], t2[:, 0], t2[:, 1], OP.add)
                    uv.append(u)

                pr = cb.tile([P, jc, R], F16, tag=f"pr{jc}")
                nc.vector.tensor_tensor(pr[:], uv[0][:], uv[1][:], OP.mult)
                nc.vector.tensor_reduce(
                    ysb[:, jo : jo + jc],
                    pr[:],
                    mybir.AxisListType.X,
                    OP.add,
                )
                # stream the output back in halves so only the last sliver
                # of writeback sits after the final reduce
                if jo + jc == J // 2:
                    nc.sync.dma_start(y_pm[:, 0 : J // 2], ysb[:, 0 : J // 2])

            nc.sync.dma_start(y_pm[:, J // 2 : J], ysb[:, J // 2 : J])

    nc.finalize()
    return nc


def _pack_table(T):
    """T [128, 128, 16] f32 -> [16384, 128] f16, 4-corner packed + pad."""
    ar = np.arange(N)
    out = np.zeros((N, N, ES), dtype=np.float16)
    for ci, (dn, dm) in enumerate(((0, 0), (0, 1), (1, 0), (1, 1))):
        rn = np.minimum(ar + dn, N - 1)
        rm = np.minimum(ar + dm, N - 1)
        out[:, :, ci * R : ci * R + R] = T[np.ix_(rn, rm)].astype(np.float16)
    return np.ascontiguousarray(out.reshape(TE, ES))


def _prep_inputs(x, core0, core1, core2, core3):
    """Host-side input marshalling: shard x over cores, lay out tensors in
    the on-chip layouts the kernel expects, and precompute the 4-corner
    packed f16 pair tables (weight preprocessing, O(N^2 R^2))."""
    xs = np.ascontiguousarray(np.asarray(x, dtype=np.float32).reshape(NCORES, BS, 4))

    core0 = np.asarray(core0, dtype=np.float32)
    core1 = np.asarray(core1, dtype=np.float32)
    core2 = np.asarray(core2, dtype=np.float32)
    core3 = np.asarray(core3, dtype=np.float32)

    # G[n0, n1, k] = sum_c core0[0, n0, c] core1[c, n1, k]
    G = np.einsum("nc,cmk->nmk", core0[0], core1, optimize=True)
    # H[n3, n2, k] = sum_c core3[c, n3, 0] core2[k, n2, c]
    H = np.einsum("cn,kmc->nmk", core3[:, :, 0], core2, optimize=True)
    tab = np.concatenate([_pack_table(G), _pack_table(H)], axis=0)

    in_maps = []
    for c in range(NCORES):
        xc_ = xs[c]
        x_pm = np.ascontiguousarray(
            xc_.reshape(J, P, 4).transpose(1, 0, 2)
        )  # [128, 256, 4]
        # coordinate pairs in wrapped list order: point i of chunk ch sits
        # at list position [i%16, i//16]; per chunk a G block then H block;
        # chunks concatenated along cols, then rows split into 8 blocks.
        blocks = []
        for ch in range(NCH):
            jc = JCS[ch]
            pts = xc_[128 * JOFF[ch] : 128 * (JOFF[ch] + jc)]
            for dims in ((0, 1), (3, 2)):
                pb = pts[:, dims].reshape(8 * jc, 16, 2).transpose(1, 0, 2)
                blocks.append(pb)  # [16, 8*jc, 2]
        flat = np.concatenate(blocks, axis=1)  # [16, 4096, 2]
        xqa = (
            flat.reshape(16, NB, CB, 2)
            .transpose(1, 0, 2, 3)
            .reshape(NB * 16, CB, 2)
        )  # [128, 512, 2]
        # mini tile: block 0 in rows 0-15, duplicated into rows 16-31
        xq0 = np.concatenate([xqa[0:16], xqa[0:16]], axis=0)  # [32, 512, 2]
        in_maps.append(
            {
                "x_pm": x_pm,
                "xq": np.ascontiguousarray(xqa),
                "xq0": np.ascontiguousarray(xq0),
                "tab": tab,
            }
        )
    return in_maps


def kernel(x, core0, core1, core2, core3):
    global _CACHED
    if _CACHED is None:
        _CACHED = _build_nc()
    nc = _CACHED
    in_maps = _prep_inputs(x, core0, core1, core2, core3)
    res = run_bass_kernel_spmd(nc, in_maps, core_ids=list(range(NCORES)))
    outs = []
    for c in range(NCORES):
        y_pm = res.results[c]["y_pm"]          # [128, 256]
        outs.append(np.ascontiguousarray(np.asarray(y_pm).T).reshape(-1))
    return np.concatenate(outs).astype(np.float32)


# revision 15
# speedup vs baseline: 1.4118x; 1.0723x over previous
"""Trainium2 Bass kernel for nn_ModelConTT_46016279609475 (TT interpolation).

y[b] = v0[b]^T V1[b] V2[b] v3[b], where v_i are linearly-interpolated slices
of tiny TT cores at per-point grid coordinates derived from x[b, :].

Strategy (per NeuronCore, data-parallel over B):
  * The joint pair tables are built ON HOST (weight preprocessing, O(N^2 R^2)
    one-time work independent of B):
      G[n0, n1, k] = sum_c core0[n0, c] * core1[c, n1, k]      (u-side)
      H[n3, n2, k] = sum_c core3[c, n3] * core2[k, n2, c]      (v-side)
    packed 4-corner per cell in f16 (64 useful + 64 pad values = 256 B, the
    minimum legal dma_gather element):
      TAB[(n*128+m), (dn*2+dm)*16 + k] = T[n+dn, m+dm, k]
    so one dma_gather element fetches everything needed for the bilinear
    interpolation of u[b] (and same for v[b]).
  * Device: compute per-point cell ids (int16) + corner weights (f16) from x,
    then 2 dma_gathers per chunk (one per table) and a DVE combine:
      y[b] = sum_k (sum_c wG_c gG[c,k]) * (sum_c wH_c gH[c,k])

Batch mapping per core: shard b of size 32768; on-chip layout is "p-minor":
element i lives at partition i%128, free col i//128, matching dma_gather's
output layout dst[i%128, i//128]. Index lists are mod-16 wrapped as
dma_gather requires (idx for i at [i%16, i//16]) and replicated to rows
16-31 (the queue's core pair); rows 32-127 are zeroed on the Pool engine so
the full-height index AP passes validation.

Pipeline startup: chunk sizes are [8,8] + [16]*14 + [8,8] output columns.
A "mini" index chain (DVE) computes chunks 0-2 from a small duplicated-row
coordinate tile straight into LL rows 0-31 (no marshal DMAs), so the first
gathers launch ~12 us in. The main chain for chunks 3-17 runs on the
otherwise-idle GpSimd engine and its lists are marshalled into wrapped
layout by small per-block DMAs during the first chunks' gather runway.
Small first/last chunks shorten pipeline fill and drain.

Exact-floor trick (f32-safe): t = (xc + 2^23) - 2^23 rounds to nearest;
g = (t > xc); floor = t - g; frac = xc - floor computed via the exact
(t1 - 2^23) path to avoid re-rounding.
"""

import numpy as np

import concourse.bass as bass
import concourse.bacc as bacc
import concourse.mybir as mybir
import concourse.tile as tile
from concourse import library_config
from concourse.bass_utils import run_bass_kernel_spmd

F32 = mybir.dt.float32
F16 = mybir.dt.float16
I16 = mybir.dt.int16
OP = mybir.AluOpType
AF = mybir.ActivationFunctionType

NCORES = 8
B = 262144
BS = B // NCORES          # 32768 points per core
P = 128                   # partitions
J = BS // P               # 256 free cols per partition
JCS = [8, 8] + [16] * 14 + [8, 4, 4]  # output cols per chunk (sum = 256)
NCH = len(JCS)
JOFF = np.cumsum([0] + JCS).tolist()
COFF = [16 * o for o in JOFF]       # int16 list-col offset per chunk
N = 128                   # mode size
R = 16                    # TT rank
TE = N * N                # table entries per side
ES = 128                  # f16 elems per gather element (64 useful + 64 pad)
MAGIC = float(2 ** 23)
SCALE = (N - 1) / 2.0     # 63.5
NB = 8                    # row-blocks in the coordinate tile
CB = 2 * BS // 16 // NB   # 512 pair-cols per row (both sides, per block)
LCOLS = 16 * J            # 4096 total int16 list cols
MINI_CH = 3               # chunks covered by the mini chain (= block 0)

_CACHED = None


def _build_nc():
    nc = bacc.Bacc("TRN2")

    x_pm = nc.dram_tensor("x_pm", [P, J, 4], F32, kind="ExternalInput")
    xq = nc.dram_tensor("xq", [P, CB, 2], F32, kind="ExternalInput")
    xq0 = nc.dram_tensor("xq0", [32, CB, 2], F32, kind="ExternalInput")
    tab = nc.dram_tensor("tab", [2 * TE, ES], F16, kind="ExternalInput")
    y_pm = nc.dram_tensor("y_pm", [P, J], F32, kind="ExternalOutput")

    tabG = tab[0:TE, :]
    tabH = tab[TE : 2 * TE, :]

    with tile.TileContext(nc) as tc:
        # all pools stay open for the whole kernel: a pool close emits a
        # release barrier that would serialize the gather loop behind the
        # entire preamble (SBUF is plentiful here, ~105 KB/partition peak)
        with (
            tc.tile_pool(name="per", bufs=1) as pe,
            tc.tile_pool(name="wp", bufs=1) as wp,
            tc.tile_pool(name="gbuf", bufs=4) as gb,
            tc.tile_pool(name="cbuf", bufs=2) as cb,
        ):
            nc.gpsimd.load_library(library_config.mlp)

            # persistent tiles. LL holds both tables' index lists as flat
            # columns; chunk ch occupies cols [COFF[ch], COFF[ch+1]) with a
            # contiguous G block then an H block.
            LL = pe.tile([P, LCOLS], I16)
            nc.gpsimd.memset(LL[:], 0)
            Wg = pe.tile([P, 4, J], F16)
            Wh = pe.tile([P, 4, J], F16)
            ysb = pe.tile([P, J], F32)
            engs = (nc.sync, nc.scalar)

            # -------- mini index chain: chunks 0-2 straight into LL -------
            # xq0 rows 0-15 hold block-0 coordinate pairs in wrapped list
            # order; rows 16-31 are a host-made duplicate, so one int16
            # store covers LL rows 0-31 with no marshal/replica DMA and the
            # first gathers launch while the main chain still runs.
            xq0_s = wp.tile([32, 2 * CB], F32)
            nc.sync.dma_start(xq0_s[:], xq0[:].rearrange("p a b -> p (a b)"))
            nc.scalar.activation(
                xq0_s[:], xq0_s[:], AF.Copy, bias=SCALE, scale=SCALE
            )
            # floor(xc) == round-to-nearest-even(xc - 0.5) here: xc - 0.5 is
            # exact in f32 over [0, 128) and the only tie (xc == 0 exactly)
            # rounds to 0 == floor. The i16 convert is fused into the op's
            # output dtype.
            lo0 = wp.tile([32, 2 * CB], I16)
            nc.vector.tensor_scalar(lo0[:], xq0_s[:], 1.0, -0.5, OP.mult, OP.add)
            lo0_pair = lo0[:].rearrange("p (m two) -> p m two", two=2)
            nc.vector.scalar_tensor_tensor(
                LL[0:32, 0:CB],
                lo0_pair[:, :, 0],
                128.0,
                lo0_pair[:, :, 1],
                OP.mult,
                OP.add,
            )

            # -------- main index chain (chunks 3-17) on GpSimd ----------
            # keeps the DVE free for the mini chain + weights; GpSimd is
            # otherwise idle until descriptor generation ramps.
            xq_s = wp.tile([P, 2 * CB], F32)
            nc.sync.dma_start(xq_s[:], xq[:].rearrange("p a b -> p (a b)"))
            nc.scalar.activation(
                xq_s[:], xq_s[:], AF.Copy, bias=SCALE, scale=SCALE
            )
            loq = wp.tile([P, 2 * CB], I16)
            nc.vector.tensor_scalar(loq[:], xq_s[:], 1.0, -0.5, OP.mult, OP.add)
            # idx = lo_hi*128 + lo_lo (int16 arithmetic, max 16383)
            lo_pair = loq[:].rearrange("p (m two) -> p m two", two=2)
            stage = wp.tile([P, CB], I16)
            nc.vector.scalar_tensor_tensor(
                stage[:],
                lo_pair[:, :, 0],
                128.0,
                lo_pair[:, :, 1],
                OP.mult,
                OP.add,
            )
            # marshal row-blocks 1-7 (chunks 3-17) into LL rows 0-15 and
            # replicate each to rows 16-31, earliest chunks first, DMAs
            # spread across the SP and Act queues.
            for i, rb in enumerate(range(1, NB)):
                cols = slice(CB * rb, CB * (rb + 1))
                engs[i % 2].dma_start(
                    LL[0:16, cols], stage[16 * rb : 16 * rb + 16, :]
                )
                engs[(i + 1) % 2].dma_start(LL[16:32, cols], LL[0:16, cols])

            # ---------------- interp weights (DVE) ----------------
            # x_s is [128, (256 j, 4 d)]; w = frac(xc), a = 1 - w, stored
            # planar f16 in aw = [128, (a|w), 4 d, 256 j] so downstream
            # ops keep stride-1 innermost dims (DVE 2x/4x perf modes).
            x_s = wp.tile([P, J * 4], F32)
            nc.scalar.dma_start(x_s[:], x_pm[:].rearrange("p a b -> p (a b)"))
            nc.scalar.activation(x_s[:], x_s[:], AF.Copy, bias=SCALE, scale=SCALE)
            low = wp.tile([P, J * 4], I16)
            nc.vector.tensor_scalar(low[:], x_s[:], 1.0, -0.5, OP.mult, OP.add)
            lof = wp.tile([P, J * 4], F32)
            nc.vector.tensor_copy(lof[:], low[:])
            aw = wp.tile([P, 2, 4, J], F16)
            # w = xc - floor(xc); write planar-transposed
            xv = x_s[:].rearrange("p (j d) -> p d j", d=4)
            fv = lof[:].rearrange("p (j d) -> p d j", d=4)
            nc.vector.tensor_tensor(aw[:, 1], xv, fv, OP.subtract)
            # a = 1 - w (f16 all through, 4x-eligible)
            nc.vector.tensor_scalar(
                aw[:, 0], aw[:, 1], -1.0, 1.0, OP.mult, OP.add
            )
            # corner products: c order = (hi,lo) in {(a,a),(a,w),(w,a),(w,w)}
            # G uses dims (0 hi, 1 lo); H uses dims (3 hi, 2 lo).
            for W, dhi, dlo in ((Wg, 0, 1), (Wh, 3, 2)):
                in1 = aw[:, :, dlo, :]  # [P, (a|w), J]
                for ci, sel in ((0, 0), (2, 1)):
                    in0 = aw[:, sel, dhi, :].unsqueeze(1).broadcast_to(
                        [P, 2, J]
                    )
                    nc.vector.tensor_tensor(
                        W[:, ci : ci + 2, :], in0, in1, OP.mult
                    )

            # ---------------- gather + combine ----------------
            for ch in range(NCH):
                jc = JCS[ch]
                nidx = P * jc
                gG = gb.tile([P, jc, ES], F16, tag=f"gG{jc}")
                nc.gpsimd.dma_gather(
                    gG[:],
                    tabG,
                    LL[:, COFF[ch] : COFF[ch] + 8 * jc],
                    nidx,
                    nidx,
                    ES,
                    queue_num=0,
                    single_packet=False,
                )
                gH = gb.tile([P, jc, ES], F16, tag=f"gH{jc}")
                nc.gpsimd.dma_gather(
                    gH[:],
                    tabH,
                    LL[:, COFF[ch] + 8 * jc : COFF[ch + 1]],
                    nidx,
                    nidx,
                    ES,
                    queue_num=0,
                    single_packet=False,
                )

                jo = JOFF[ch]
                uv = []
                for ti, (g, W) in enumerate(((gG, Wg), (gH, Wh))):
                    # m[c, j, k] = gathered corner value * corner weight
                    # (weight broadcast over k via stride-0 AP)
                    m = cb.tile([P, 4, jc, R], F16, tag=f"m{ti}{jc}")
                    gv2 = g[:, :, 0 : 4 * R].rearrange(
                        "p j (c k) -> p c j k", k=R
                    )
                    wbc = (
                        W[:, :, jo : jo + jc]
                        .unsqueeze(3)
                        .broadcast_to([P, 4, jc, R])
                    )
                    nc.vector.tensor_tensor(m[:], gv2, wbc, OP.mult)
                    t2 = cb.tile([P, 2, jc, R], F16, tag=f"t{ti}{jc}")
                    nc.vector.tensor_tensor(
                        t2[:], m[:, 0:2], m[:, 2:4], OP.add
                    )
                    u = cb.tile([P, jc, R], F16, tag=f"u{ti}{jc}")
                    nc.vector.tensor_tensor(u[:], t2[:, 0], t2[:, 1], OP.add)
                    uv.append(u)

                pr = cb.tile([P, jc, R], F16, tag=f"pr{jc}")
                nc.vector.tensor_tensor(pr[:], uv[0][:], uv[1][:], OP.mult)
                nc.vector.tensor_reduce(
                    ysb[:, jo : jo + jc],
                    pr[:],
                    mybir.AxisListType.X,
                    OP.add,
                )
                # stream the output back in halves so only the last sliver
                # of writeback sits after the final reduce
                if jo + jc == J // 2:
                    nc.sync.dma_start(y_pm[:, 0 : J // 2], ysb[:, 0 : J // 2])

            nc.sync.dma_start(y_pm[:, J // 2 : J], ysb[:, J // 2 : J])

    nc.finalize()
    return nc


def _pack_table(T):
    """T [128, 128, 16] f32 -> [16384, 128] f16, 4-corner packed + pad."""
    ar = np.arange(N)
    out = np.zeros((N, N, ES), dtype=np.float16)
    for ci, (dn, dm) in enumerate(((0, 0), (0, 1), (1, 0), (1, 1))):
        rn = np.minimum(ar + dn, N - 1)
        rm = np.minimum(ar + dm, N - 1)
        out[:, :, ci * R : ci * R + R] = T[np.ix_(rn, rm)].astype(np.float16)
    return np.ascontiguousarray(out.reshape(TE, ES))


def _prep_inputs(x, core0, core1, core2, core3):
    """Host-side input marshalling: shard x over cores, lay out tensors in
    the on-chip layouts the kernel expects, and precompute the 4-corner
    packed f16 pair tables (weight preprocessing, O(N^2 R^2))."""
    xs = np.ascontiguousarray(np.asarray(x, dtype=np.float32).reshape(NCORES, BS, 4))

    core0 = np.asarray(core0, dtype=np.float32)
    core1 = np.asarray(core1, dtype=np.float32)
    core2 = np.asarray(core2, dtype=np.float32)
    core3 = np.asarray(core3, dtype=np.float32)

    # G[n0, n1, k] = sum_c core0[0, n0, c] core1[c, n1, k]
    G = np.einsum("nc,cmk->nmk", core0[0], core1, optimize=True)
    # H[n3, n2, k] = sum_c core3[c, n3, 0] core2[k, n2, c]
    H = np.einsum("cn,kmc->nmk", core3[:, :, 0], core2, optimize=True)
    tab = np.concatenate([_pack_table(G), _pack_table(H)], axis=0)

    in_maps = []
    for c in range(NCORES):
        xc_ = xs[c]
        x_pm = np.ascontiguousarray(
            xc_.reshape(J, P, 4).transpose(1, 0, 2)
        )  # [128, 256, 4]
        # coordinate pairs in wrapped list order: point i of chunk ch sits
        # at list position [i%16, i//16]; per chunk a G block then H block;
        # chunks concatenated along cols, then rows split into 8 blocks.
        blocks = []
        for ch in range(NCH):
            jc = JCS[ch]
            pts = xc_[128 * JOFF[ch] : 128 * (JOFF[ch] + jc)]
            for dims in ((0, 1), (3, 2)):
                pb = pts[:, dims].reshape(8 * jc, 16, 2).transpose(1, 0, 2)
                blocks.append(pb)  # [16, 8*jc, 2]
        flat = np.concatenate(blocks, axis=1)  # [16, 4096, 2]
        xqa = (
            flat.reshape(16, NB, CB, 2)
            .transpose(1, 0, 2, 3)
            .reshape(NB * 16, CB, 2)
        )  # [128, 512, 2]
        # mini tile: block 0 in rows 0-15, duplicated into rows 16-31
        xq0 = np.concatenate([xqa[0:16], xqa[0:16]], axis=0)  # [32, 512, 2]
        in_maps.append(
            {
                "x_pm": x_pm,
                "xq": np.ascontiguousarray(xqa),
                "xq0": np.ascontiguousarray(xq0),
                "tab": tab,
            }
        )
    return in_maps


def kernel(x, core0, core1, core2, core3):
    global _CACHED
    if _CACHED is None:
        _CACHED = _build_nc()
    nc = _CACHED
    in_maps = _prep_inputs(x, core0, core1, core2, core3)
    res = run_bass_kernel_spmd(nc, in_maps, core_ids=list(range(NCORES)))
    outs = []
    for c in range(NCORES):
        y_pm = res.results[c]["y_pm"]          # [128, 256]
        outs.append(np.ascontiguousarray(np.asarray(y_pm).T).reshape(-1))
    return np.concatenate(outs).astype(np.float32)
